# revision 27
# baseline (speedup 1.0000x reference)
import numpy as np
from contextlib import ExitStack

import concourse.bass as bass
import concourse.tile as tile
import concourse.mybir as mybir
from concourse import bacc
from concourse.bass_utils import run_bass_kernel_spmd

B, T, D, H, DH, K = 32, 512, 512, 8, 64, 64
NCORES = 8
BL = B // NCORES          # 4 batches per core
NT = T // 128             # 4 t-tiles
ND = D // 128             # 4 d-tiles
NJ = 16                   # FFN hidden tiles (2048/128)
F32 = mybir.dt.float32
F32R = mybir.dt.float32r
AF = mybir.ActivationFunctionType
ALU = mybir.AluOpType
AX = mybir.AxisListType.X

MM_DT = mybir.dt.float32r  # fast fp32r matmuls (4x PE throughput at N>=256)


def _mm(nc, out, lhsT, rhs, start, stop):
    n = out.shape[-1]
    dt_ = MM_DT if (MM_DT is not None and n >= 256) else F32
    lhsT = lhsT.bitcast(dt_)
    rhs = rhs.bitcast(dt_)
    nc.tensor.matmul(out, lhsT, rhs, start=start, stop=stop)


def _patch_act_tables():
    # The act-table-load pass picks the FIRST set containing each function,
    # so exp->exp_and_others and ln->natural_log ping-pong (2.7us per load).
    # Hide exp/ln from every set except natural_log_exp_and_others so both
    # resolve to the one set that holds them jointly. Safe: ids stay the
    # act_info.json indices; we only narrow the pass's choice.
    from concourse import hw_specs
    orig = hw_specs.get_activation_tables
    if getattr(bacc, "_act_tables_patched", False):
        return
    def patched(arch):
        out = {}
        for name, fns in orig(arch).items():
            fns = set(fns)
            if name != "natural_log_exp_and_others":
                fns.discard(AF.Exp)
                fns.discard(AF.Ln)
            out[name] = fns
        return out
    bacc.get_activation_tables = patched
    bacc._act_tables_patched = True


def _build():
    _patch_act_tables()
    nc = bacc.Bacc("TRN2", target_bir_lowering=False, debug=False,
                   num_devices=NCORES)
    dt = nc.dram_tensor
    # inputs (per core shapes)
    x_d = dt("x", [BL, T, D], F32, kind="ExternalInput")
    keys_d = dt("keys", [BL, K, D], F32, kind="ExternalInput")
    keysT_d = dt("keysT", [BL, D, K], F32R, kind="ExternalInput")
    vals_d = dt("vals", [BL, K, D], F32R, kind="ExternalInput")
    age_d = dt("age", [BL, K], F32, kind="ExternalInput")
    str_d = dt("strength", [BL, K], F32, kind="ExternalInput")
    wq_d = dt("Wq", [D, D], F32R, kind="ExternalInput")
    wk_d = dt("Wk", [D, D], F32R, kind="ExternalInput")
    wv_d = dt("Wv", [D, D], F32R, kind="ExternalInput")
    wo_d = dt("Wo", [D, D], F32R, kind="ExternalInput")
    bq_d = dt("bq", [D], F32, kind="ExternalInput")
    bk_d = dt("bk", [D], F32, kind="ExternalInput")
    bvb_d = dt("bv_bcast", [128, D], F32, kind="ExternalInput")
    bo_d = dt("bo", [D], F32, kind="ExternalInput")
    w1_d = dt("W1", [D, 8 * D], F32R, kind="ExternalInput")
    w2_d = dt("W2", [4 * D, D], F32R, kind="ExternalInput")
    wwk_d = dt("Wwk", [D, D], F32R, kind="ExternalInput")
    wwv_d = dt("Wwv", [D, D], F32R, kind="ExternalInput")
    bwk_d = dt("bwk", [D], F32, kind="ExternalInput")
    bwv_d = dt("bwv", [D], F32, kind="ExternalInput")
    wws_d = dt("Wws", [D], F32R, kind="ExternalInput")
    bws_d = dt("bws", [1, 1], F32, kind="ExternalInput")
    wg1_d = dt("Wg1", [D + 2, D], F32R, kind="ExternalInput")
    bg1_d = dt("bg1", [1, D], F32, kind="ExternalInput")
    wg2_d = dt("Wg2", [D], F32R, kind="ExternalInput")
    bg2_d = dt("bg2", [1, 1], F32, kind="ExternalInput")
    rms1_d = dt("rms1", [D], F32, kind="ExternalInput")
    rmskv_d = dt("rms_kv", [D], F32, kind="ExternalInput")
    rms2_d = dt("rms2", [D], F32, kind="ExternalInput")
    rmsrd_d = dt("rms_read", [1, D], F32, kind="ExternalInput")
    cos_d = dt("rope_cos", [128, T], F32, kind="ExternalInput")
    sin_d = dt("rope_sin", [128, T], F32, kind="ExternalInput")
    mask_d = dt("mask", [T, T], F32, kind="ExternalInput")
    id_d = dt("ident", [128, 128], F32, kind="ExternalInput")
    ones_d = dt("ones", [128, 128], F32R, kind="ExternalInput")
    swap_d = dt("swapm", [128, 128], F32R, kind="ExternalInput")
    # outputs
    out_d = dt("out", [BL, T, D], F32, kind="ExternalOutput")
    kn_d = dt("keys_new", [BL, K, D], F32, kind="ExternalOutput")
    vn_d = dt("vals_new", [BL, K, D], F32, kind="ExternalOutput")
    an_d = dt("age_new", [BL, K], F32, kind="ExternalOutput")
    sn_d = dt("strength_new", [BL, K], F32, kind="ExternalOutput")
    gate_d = dt("gate", [BL, 1], F32, kind="ExternalOutput")

    def colview(d):  # [512] dram -> [128, 4] col tile view
        return d.ap().rearrange("(j p) -> p j", p=128)

    with tile.TileContext(nc) as tc, ExitStack() as octx, \
            nc.allow_low_precision(reason="fp32r matmul operands"):
        cst = octx.enter_context(tc.tile_pool(name="cst", bufs=1))
        ps = octx.enter_context(tc.tile_pool(name="ps", bufs=7, space="PSUM"))

        ident = cst.tile([128, 128], F32, tag="ident")
        nc.sync.dma_start(ident[:], id_d.ap())
        ones = cst.tile([128, 128], F32R, tag="ones")
        nc.sync.dma_start(ones[:], ones_d.ap())
        swapm = cst.tile([128, 128], F32R, tag="swapm")
        nc.sync.dma_start(swapm[:], swap_d.ap())
        cosT = cst.tile([128, T], F32, tag="cos")
        nc.sync.dma_start(cosT[:], cos_d.ap())
        sinT = cst.tile([128, T], F32, tag="sin")
        nc.sync.dma_start(sinT[:], sin_d.ap())
        bq_c = cst.tile([128, ND], F32, tag="bqc")
        nc.sync.dma_start(bq_c[:], colview(bq_d))
        bk_c = cst.tile([128, ND], F32, tag="bkc")
        nc.sync.dma_start(bk_c[:], colview(bk_d))
        bo_c = cst.tile([128, ND], F32, tag="boc")
        nc.sync.dma_start(bo_c[:], colview(bo_d))
        rms1_c = cst.tile([128, ND], F32, tag="rms1c")
        nc.sync.dma_start(rms1_c[:], colview(rms1_d))
        rmskv_c = cst.tile([128, ND], F32, tag="rmskvc")
        nc.sync.dma_start(rmskv_c[:], colview(rmskv_d))
        rms2_c = cst.tile([128, ND], F32, tag="rms2c")
        nc.sync.dma_start(rms2_c[:], colview(rms2_d))
        wws_c = cst.tile([128, ND], F32R, tag="wwsc")
        nc.sync.dma_start(wws_c[:], colview(wws_d))
        wg2_c = cst.tile([128, ND], F32R, tag="wg2c")
        nc.sync.dma_start(wg2_c[:], colview(wg2_d))
        rmsrd_r = cst.tile([1, D], F32, tag="rmsrdr")
        nc.sync.dma_start(rmsrd_r[:], rmsrd_d.ap())
        bg1_r = cst.tile([1, D], F32, tag="bg1r")
        nc.sync.dma_start(bg1_r[:], bg1_d.ap())
        bwk_r = cst.tile([1, D], F32, tag="bwkr")
        nc.sync.dma_start(bwk_r[:], bwk_d.ap().rearrange("(o d) -> o d", o=1))
        bwv_r = cst.tile([1, D], F32, tag="bwvr")
        nc.sync.dma_start(bwv_r[:], bwv_d.ap().rearrange("(o d) -> o d", o=1))
        bws_s = cst.tile([1, 1], F32, tag="bwss")
        nc.sync.dma_start(bws_s[:], bws_d.ap())
        bg2_s = cst.tile([1, 1], F32, tag="bg2s")
        nc.sync.dma_start(bg2_s[:], bg2_d.ap())
        bvb = cst.tile([128, D], F32, tag="bvb")
        nc.sync.dma_start(bvb[:], bvb_d.ap())
        wg1t_r = cst.tile([2, D], F32R, tag="wg1t")
        nc.sync.dma_start(wg1t_r[:], wg1_d.ap()[D:D + 2, :])
        eps_c = cst.tile([128, 1], F32, tag="epsc")
        nc.vector.memset(eps_c[:], 1e-6)

        h1p = octx.enter_context(tc.tile_pool(name="h1p", bufs=BL * ND))
        h1T = [[h1p.tile([128, T], F32, tag="h1", name=f"h1_{b}_{j}")
                for j in range(ND)] for b in range(BL)]

        def ln_exp_scale(out_ap, in_ap, ln_scale, ln_bias, exp_scale):
            # out = exp(exp_scale * ln(ln_scale*in + ln_bias))
            t = small.tile(list(in_ap.shape), F32, tag="lnexp", name="lnexp_t")
            bias = eps_c[0:in_ap.shape[0], 0:1] if ln_bias else 0.0
            nc.scalar.activation(t[:], in_ap, AF.Ln, scale=ln_scale,
                                 bias=bias)
            nc.scalar.activation(out_ap, t[:], AF.Exp, scale=exp_scale)

        # ---------------- Phase A: attention -> h1T ----------------
        with ExitStack() as actx:
            wpool = actx.enter_context(tc.tile_pool(name="wA", bufs=1))
            apool = actx.enter_context(tc.tile_pool(name="actA", bufs=4))
            epool = actx.enter_context(tc.tile_pool(name="eA", bufs=3))
            spool = actx.enter_context(tc.tile_pool(name="sA", bufs=4))
            small = actx.enter_context(tc.tile_pool(name="smA", bufs=6))

            wq = [wpool.tile([128, D], F32R, tag=f"wq{i}", name=f"wq{i}") for i in range(ND)]
            wk = [wpool.tile([128, D], F32R, tag=f"wk{i}", name=f"wk{i}") for i in range(ND)]
            wv = [wpool.tile([128, D], F32R, tag=f"wv{i}", name=f"wv{i}") for i in range(ND)]
            wo = [wpool.tile([128, D], F32R, tag=f"wo{i}", name=f"wo{i}") for i in range(ND)]
            for i in range(ND):
                nc.sync.dma_start(wq[i][:], wq_d.ap()[128 * i:128 * (i + 1), :])
                nc.sync.dma_start(wk[i][:], wk_d.ap()[128 * i:128 * (i + 1), :])
                nc.sync.dma_start(wv[i][:], wv_d.ap()[128 * i:128 * (i + 1), :])
                nc.sync.dma_start(wo[i][:], wo_d.ap()[128 * i:128 * (i + 1), :])
            maskt = [wpool.tile([128, T], F32, tag=f"mask{i}", name=f"mask{i}") for i in range(NT)]
            for i in range(NT):
                nc.sync.dma_start(maskt[i][:],
                                  mask_d.ap()[128 * i:128 * (i + 1), :])

            for b in range(BL):
                # load x, rms stats
                xt, rcol = [], []
                for i in range(NT):
                    t = apool.tile([128, D], F32, tag="x")
                    nc.sync.dma_start(t[:], x_d.ap()[b, 128 * i:128 * (i + 1), :])
                    xt.append(t)
                for i in range(NT):
                    sq = spool.tile([128, D], F32, tag="sq")
                    rc = small.tile([128, 1], F32, tag="rc")
                    nc.vector.scalar_tensor_tensor(
                        sq[:], xt[i][:], 1.0, xt[i][:], ALU.mult, ALU.mult,
                        accum_out=rc[:])
                    r = small.tile([128, 1], F32, tag="rr")
                    ln_exp_scale(r[:], rc[:], 1.0 / D, 1e-6, -0.5)
                    rcol.append(r)
                # transpose x -> xT
                xT = []
                for j in range(ND):
                    pt = ps.tile([128, T], F32, tag="ps")
                    for i in range(NT):
                        nc.tensor.transpose(
                            pt[:, 128 * i:128 * (i + 1)],
                            xt[i][:, 128 * j:128 * (j + 1)], ident[:])
                    t = apool.tile([128, T], F32, tag="xT")
                    nc.scalar.copy(t[:], pt[:])
                    xT.append(t)
                # xn in place of x, then transpose -> xnT
                for i in range(NT):
                    nc.vector.tensor_scalar(xt[i][:], xt[i][:], rcol[i][:],
                                            None, ALU.mult)
                xnT = []
                for j in range(ND):
                    pt = ps.tile([128, T], F32, tag="ps")
                    for i in range(NT):
                        nc.tensor.transpose(
                            pt[:, 128 * i:128 * (i + 1)],
                            xt[i][:, 128 * j:128 * (j + 1)], ident[:])
                    t = apool.tile([128, T], F32R, tag="xnT")
                    nc.scalar.copy(t[:], pt[:])
                    xnT.append(t)
                nkvT = []
                for j in range(ND):
                    t2 = apool.tile([128, T], F32R, tag="nkvT")
                    nc.vector.tensor_scalar(t2[:], xnT[j][:],
                                            rmskv_c[:, j:j + 1], None, ALU.mult)
                    nkvT.append(t2)
                nxT = xnT  # nxT in place of xnT
                for j in range(ND):
                    nc.vector.tensor_scalar(nxT[j][:], xnT[j][:],
                                            rms1_c[:, j:j + 1], None, ALU.mult)
                # projections
                qT, kT, vN = [], [], []
                for j in range(ND):
                    pq = ps.tile([128, T], F32, tag="ps")
                    for i in range(ND):
                        _mm(nc, pq[:], wq[i][:, 128 * j:128 * (j + 1)],
                            nxT[i][:], i == 0, i == ND - 1)
                    t = apool.tile([128, T], F32R, tag="qT")
                    nc.scalar.activation(t[:], pq[:], AF.Identity,
                                         bias=bq_c[:, j:j + 1])
                    qT.append(t)
                    pk = ps.tile([128, T], F32, tag="ps")
                    for i in range(ND):
                        _mm(nc, pk[:], wk[i][:, 128 * j:128 * (j + 1)],
                            nkvT[i][:], i == 0, i == ND - 1)
                    t = apool.tile([128, T], F32R, tag="kT")
                    nc.scalar.activation(t[:], pk[:], AF.Identity,
                                         bias=bk_c[:, j:j + 1])
                    kT.append(t)
                for i in range(NT):
                    pv = ps.tile([128, D], F32, tag="ps")
                    for kk in range(ND):
                        _mm(nc, pv[:], nkvT[kk][:, 128 * i:128 * (i + 1)],
                            wv[kk][:], kk == 0, kk == ND - 1)
                    t = apool.tile([128, D], F32R, tag="vN")
                    nc.vector.tensor_tensor(t[:], pv[:], bvb[:], ALU.add)
                    vN.append(t)
                # rope on qT, kT (in place): rot = q*cos + swap(q)*sin
                for lst in (qT, kT):
                    for j in range(ND):
                        psw = ps.tile([128, T], F32, tag="ps")
                        _mm(nc, psw[:], swapm[:], lst[j][:], True, True)
                        qc = spool.tile([128, T], F32, tag="ropeC")
                        nc.vector.tensor_tensor(qc[:], lst[j][:], cosT[:],
                                                ALU.mult)
                        tm = spool.tile([128, T], F32, tag="ropeS")
                        nc.vector.tensor_tensor(tm[:], psw[:], sinT[:],
                                                ALU.mult)
                        nc.vector.tensor_tensor(lst[j][:], qc[:], tm[:],
                                                ALU.add)
                # attention heads
                oT = [apool.tile([128, T], F32R, tag="oT", name=f"oT{j}") for j in range(ND)]
                for h in range(H):
                    jt, pr = h // 2, (h % 2) * 64
                    pcs = ps.tile([1, T], F32, tag="ps")
                    po = ps.tile([64, T], F32, tag="ps")
                    for i in range(NT):
                        pss = ps.tile([128, T], F32, tag="ps")
                        _mm(nc, pss[:],
                            kT[jt][pr:pr + 64, 128 * i:128 * (i + 1)],
                            qT[jt][pr:pr + 64, :], True, True)
                        e = epool.tile([128, T], F32R, tag="E")
                        nc.scalar.activation(e[:], pss[:], AF.Exp,
                                             scale=1.0 / 8.0)
                        nc.gpsimd.tensor_tensor(e[:], e[:], maskt[i][:],
                                                ALU.mult)
                        _mm(nc, pcs[:], ones[:, 0:1], e[:], i == 0, i == NT - 1)
                        _mm(nc, po[:], vN[i][:, 64 * h:64 * (h + 1)], e[:],
                            i == 0, i == NT - 1)
                    rr = small.tile([1, T], F32R, tag="csr")
                    nc.vector.reciprocal(rr[:], pcs[:])
                    pbc = ps.tile([64, T], F32, tag="ps")
                    _mm(nc, pbc[:], ones[0:1, 0:64], rr[:], True, True)
                    bcs = spool.tile([64, T], F32, tag="bcs")
                    nc.scalar.copy(bcs[:], pbc[:])
                    nc.vector.tensor_tensor(oT[jt][pr:pr + 64, :], po[:],
                                            bcs[:], ALU.mult)
                # Wo projection + residual
                for j in range(ND):
                    ph = ps.tile([128, T], F32, tag="ps")
                    for kk in range(ND):
                        _mm(nc, ph[:], wo[kk][:, 128 * j:128 * (j + 1)],
                            oT[kk][:], kk == 0, kk == ND - 1)
                    nc.vector.scalar_tensor_tensor(
                        h1T[b][j][:], ph[:], bo_c[:, j:j + 1], xT[j][:],
                        ALU.add, ALU.add)

        # ---------------- Phase B: FFN (h2 accumulated into h1T) --------
        with ExitStack() as bctx:
            w1p = bctx.enter_context(tc.tile_pool(name="w1p", bufs=8))
            w2p = bctx.enter_context(tc.tile_pool(name="w2p", bufs=8))
            hnp = bctx.enter_context(tc.tile_pool(name="hnp", bufs=BL * ND))
            bsc = bctx.enter_context(tc.tile_pool(name="bsc", bufs=3))
            apb = bctx.enter_context(tc.tile_pool(name="apb", bufs=8))
            small = bctx.enter_context(tc.tile_pool(name="smB", bufs=6))

            h1nT = []
            for b in range(BL):
                # rms over d (partition dim) via ones-matmul of squares
                pssq = ps.tile([1, T], F32, tag="ps")
                for j in range(ND):
                    sq = bsc.tile([128, T], F32R, tag="sqB")
                    nc.vector.tensor_tensor(sq[:], h1T[b][j][:], h1T[b][j][:],
                                            ALU.mult)
                    _mm(nc, pssq[:], ones[:, 0:1], sq[:], j == 0, j == ND - 1)
                rrow = small.tile([1, T], F32R, tag="rrowB")
                ln_exp_scale(rrow[:], pssq[:], 1.0 / D, 1e-6, -0.5)
                prb = ps.tile([128, T], F32, tag="ps")
                _mm(nc, prb[:], ones[0:1, :], rrow[:], True, True)
                rb = bsc.tile([128, T], F32, tag="rbB")
                nc.scalar.copy(rb[:], prb[:])
                row = []
                for j in range(ND):
                    t = hnp.tile([128, T], F32R, tag="h1n")
                    nc.vector.scalar_tensor_tensor(
                        t[:], h1T[b][j][:], rms2_c[:, j:j + 1], rb[:],
                        ALU.mult, ALU.mult)
                    row.append(t)
                h1nT.append(row)

            for jg in range(4):
                js = [4 * jg + a for a in range(4)]
                w1g, w1v, w2t = {}, {}, {}
                for j in js:
                    tg = w1p.tile([128, D], F32R, tag="w1g")
                    for kk in range(ND):
                        nc.sync.dma_start(
                            tg[:, 128 * kk:128 * (kk + 1)],
                            w1_d.ap()[128 * kk:128 * (kk + 1),
                                      128 * j:128 * (j + 1)])
                    w1g[j] = tg
                    tv = w1p.tile([128, D], F32R, tag="w1v")
                    for kk in range(ND):
                        nc.sync.dma_start(
                            tv[:, 128 * kk:128 * (kk + 1)],
                            w1_d.ap()[128 * kk:128 * (kk + 1),
                                      4 * D + 128 * j:4 * D + 128 * (j + 1)])
                    w1v[j] = tv
                    t2 = w2p.tile([128, D], F32R, tag="w2")
                    nc.sync.dma_start(t2[:],
                                      w2_d.ap()[128 * j:128 * (j + 1), :])
                    w2t[j] = t2
                for b in range(BL):
                    ajb = {}
                    for j in js:
                        pg = ps.tile([128, T], F32, tag="ps")
                        pv = ps.tile([128, T], F32, tag="ps")
                        for kk in range(ND):
                            _mm(nc, pg[:],
                                w1g[j][:, 128 * kk:128 * (kk + 1)],
                                h1nT[b][kk][:], kk == 0, kk == ND - 1)
                        for kk in range(ND):
                            _mm(nc, pv[:],
                                w1v[j][:, 128 * kk:128 * (kk + 1)],
                                h1nT[b][kk][:], kk == 0, kk == ND - 1)
                        sg = bsc.tile([128, T], F32, tag="sigB")
                        nc.scalar.activation(sg[:], pg[:], AF.Silu)
                        a = apb.tile([128, T], F32R, tag="aB")
                        nc.vector.tensor_tensor(a[:], sg[:], pv[:], ALU.mult)
                        ajb[j] = a
                    for jo in range(ND):
                        ph = ps.tile([128, T], F32, tag="ps")
                        for idx, j in enumerate(js):
                            _mm(nc, ph[:],
                                w2t[j][:, 128 * jo:128 * (jo + 1)],
                                ajb[j][:], idx == 0, idx == 3)
                        nc.vector.tensor_tensor(h1T[b][jo][:], h1T[b][jo][:],
                                                ph[:], ALU.add)

        # ---------------- Phase C: memory ops + output ----------------
        with ExitStack() as cctx:
            wcp = cctx.enter_context(tc.tile_pool(name="wC", bufs=1))
            cp = cctx.enter_context(tc.tile_pool(name="actC", bufs=2))
            small = cctx.enter_context(tc.tile_pool(name="smC", bufs=2))

            wwk = [wcp.tile([128, D], F32R, tag=f"wwk{i}", name=f"wwk{i}") for i in range(ND)]
            wwv = [wcp.tile([128, D], F32R, tag=f"wwv{i}", name=f"wwv{i}") for i in range(ND)]
            wg1 = [wcp.tile([128, D], F32R, tag=f"wg1{i}", name=f"wg1{i}") for i in range(ND)]
            for i in range(ND):
                nc.sync.dma_start(wwk[i][:],
                                  wwk_d.ap()[128 * i:128 * (i + 1), :])
                nc.sync.dma_start(wwv[i][:],
                                  wwv_d.ap()[128 * i:128 * (i + 1), :])
                nc.sync.dma_start(wg1[i][:],
                                  wg1_d.ap()[128 * i:128 * (i + 1), :])

            for b in range(BL):
                h2 = h1T[b]
                # q_win cols (sum over t; 1/512 folded later)
                qwc = small.tile([128, ND], F32R, tag="qwc")
                for j in range(ND):
                    nc.vector.reduce_sum(qwc[:, j:j + 1], h2[j][:], axis=AX)
                # episodic tensors
                keysN = cp.tile([64, D], F32, tag="keysN")
                nc.sync.dma_start(keysN[:], keys_d.ap()[b])
                valsN = cp.tile([64, D], F32R, tag="valsN")
                nc.sync.dma_start(valsN[:], vals_d.ap()[b])
                keysTt = [cp.tile([128, K], F32R, tag=f"keysT{i}", name=f"keysTt{i}")
                          for i in range(ND)]
                for i in range(ND):
                    nc.sync.dma_start(keysTt[i][:],
                                      keysT_d.ap()[b, 128 * i:128 * (i + 1), :])
                age_r = small.tile([1, K], F32, tag="ager")
                nc.sync.dma_start(age_r[:], age_d.ap()[b:b + 1, :])
                str_r = small.tile([1, K], F32, tag="strr")
                nc.sync.dma_start(str_r[:], str_d.ap()[b:b + 1, :])

                # ||q_win|| -> rn scalar ; note qwc = 512*q_win
                pnq = ps.tile([1, 1], F32, tag="ps")
                for j in range(ND):
                    _mm(nc, pnq[:], qwc[:, j:j + 1], qwc[:, j:j + 1],
                        j == 0, j == ND - 1)
                nrm = small.tile([1, 1], F32, tag="nrm")
                ln_exp_scale(nrm[:], pnq[:], 1.0, 0.0, 0.5)  # = 512*||q_win||
                rn = small.tile([1, 1], F32, tag="rn")
                nc.vector.tensor_scalar(rn[:], nrm[:], 1.0 / D, 1e-6,
                                        ALU.mult, ALU.add)
                nc.vector.reciprocal(rn[:], rn[:])
                # keys row norms -> rk row
                kssq = small.tile([64, 1], F32, tag="kssq")
                scr = cp.tile([64, D], F32, tag="scrC")
                nc.vector.scalar_tensor_tensor(scr[:], keysN[:], 1.0,
                                               keysN[:], ALU.mult, ALU.mult,
                                               accum_out=kssq[:])
                knrm = small.tile([64, 1], F32, tag="knrm")
                ln_exp_scale(knrm[:], kssq[:], 1.0, 0.0, 0.5)
                nc.vector.tensor_scalar(knrm[:], knrm[:], 1e-6, None, ALU.add)
                nc.vector.reciprocal(knrm[:], knrm[:])
                prk = ps.tile([1, 64], F32, tag="ps")
                nc.tensor.transpose(prk[:], knrm[:], ident[0:64, 0:64])
                rk_r = small.tile([1, K], F32, tag="rkr")
                nc.vector.tensor_copy(rk_r[:], prk[:])
                # sim_r row
                psr = ps.tile([1, K], F32, tag="ps")
                for j in range(ND):
                    _mm(nc, psr[:], qwc[:, j:j + 1], keysTt[j][:],
                        j == 0, j == ND - 1)
                simr = small.tile([1, K], F32, tag="simr")
                nc.vector.tensor_scalar(simr[:], psr[:], rn[:], 1.0 / D,
                                        ALU.mult, ALU.mult)
                nc.vector.tensor_tensor(simr[:], simr[:], rk_r[:], ALU.mult)
                # reader logits
                lns = small.tile([1, K], F32, tag="lns")
                nc.vector.tensor_scalar(lns[:], str_r[:], 0.001, 1e9,
                                        ALU.max, ALU.min)
                nc.scalar.activation(lns[:], lns[:], AF.Ln)
                msk = small.tile([1, K], F32, tag="mskC")
                nc.vector.tensor_scalar(msk[:], str_r[:], 0.001, None,
                                        ALU.is_gt)
                nc.vector.tensor_scalar(msk[:], msk[:], 1.0, 1000.0,
                                        ALU.subtract, ALU.mult)
                lg = small.tile([1, K], F32, tag="lg")
                nc.vector.scalar_tensor_tensor(lg[:], lns[:], 0.5, simr[:],
                                               ALU.mult, ALU.add)
                nc.vector.scalar_tensor_tensor(lg[:], age_r[:], -0.02, lg[:],
                                               ALU.mult, ALU.add)
                nc.vector.tensor_tensor(lg[:], lg[:], msk[:], ALU.add)
                # softmax over K (row)
                negm = small.tile([1, 1], F32, tag="negm")
                nc.vector.reduce_max(negm[:], lg[:], axis=AX, negate=True)
                er = small.tile([1, K], F32, tag="er")
                nc.scalar.activation(er[:], lg[:], AF.Exp, bias=negm[:])
                ssum = small.tile([1, 1], F32, tag="ssum")
                nc.vector.reduce_sum(ssum[:], er[:], axis=AX)
                nc.vector.reciprocal(ssum[:], ssum[:])
                wread = small.tile([1, K], F32, tag="wread")
                nc.vector.tensor_scalar(wread[:], er[:], ssum[:], None,
                                        ALU.mult)
                pwc = ps.tile([64, 1], F32, tag="ps")
                nc.tensor.transpose(pwc[:], wread[:], ident[0:1, 0:1])
                wrc = small.tile([64, 1], F32R, tag="wrc")
                nc.vector.tensor_copy(wrc[:], pwc[:])
                prd = ps.tile([1, D], F32, tag="ps")
                _mm(nc, prd[:], wrc[:], valsN[:], True, True)
                # read = rms(read_raw) * rms_read
                rd0 = small.tile([1, D], F32, tag="rd0")
                nc.vector.tensor_copy(rd0[:], prd[:])
                rdsq = small.tile([1, 1], F32, tag="rdsq")
                rscr = small.tile([1, D], F32, tag="rscr")
                nc.vector.scalar_tensor_tensor(rscr[:], rd0[:], 1.0, rd0[:],
                                               ALU.mult, ALU.mult,
                                               accum_out=rdsq[:])
                rrd = small.tile([1, 1], F32, tag="rrd")
                ln_exp_scale(rrd[:], rdsq[:], 1.0 / D, 1e-6, -0.5)
                read_r = small.tile([1, D], F32, tag="readr")
                nc.vector.tensor_scalar(read_r[:], rd0[:], rrd[:], None,
                                        ALU.mult)
                nc.vector.tensor_tensor(read_r[:], read_r[:], rmsrd_r[:],
                                        ALU.mult)
                # write key/val rows
                pwk = ps.tile([1, D], F32, tag="ps")
                for j in range(ND):
                    _mm(nc, pwk[:], qwc[:, j:j + 1], wwk[j][:],
                        j == 0, j == ND - 1)
                wk_r = small.tile([1, D], F32R, tag="wkrow")
                nc.vector.scalar_tensor_tensor(wk_r[:], pwk[:], 1.0 / D,
                                               bwk_r[:], ALU.mult, ALU.add)
                pwv = ps.tile([1, D], F32, tag="ps")
                for j in range(ND):
                    _mm(nc, pwv[:], qwc[:, j:j + 1], wwv[j][:],
                        j == 0, j == ND - 1)
                wv_r = small.tile([1, D], F32R, tag="wvrow")
                nc.vector.scalar_tensor_tensor(wv_r[:], pwv[:], 1.0 / D,
                                               bwv_r[:], ALU.mult, ALU.add)
                # wk cols for norm + sim_w
                pkc = ps.tile([128, ND], F32, tag="ps")
                for j in range(ND):
                    nc.tensor.transpose(pkc[:, j:j + 1],
                                        wk_r[0:1, 128 * j:128 * (j + 1)]
                                        .bitcast(F32),
                                        ident[0:1, 0:1])
                wkc = small.tile([128, ND], F32R, tag="wkc")
                nc.vector.tensor_copy(wkc[:], pkc[:])
                pws = ps.tile([1, 1], F32, tag="ps")
                for j in range(ND):
                    _mm(nc, pws[:], wkc[:, j:j + 1], wkc[:, j:j + 1],
                        j == 0, j == ND - 1)
                rw = small.tile([1, 1], F32, tag="rw")
                ln_exp_scale(rw[:], pws[:], 1.0, 0.0, 0.5)
                nc.vector.tensor_scalar(rw[:], rw[:], 1e-6, None, ALU.add)
                nc.vector.reciprocal(rw[:], rw[:])
                psw = ps.tile([1, K], F32, tag="ps")
                for j in range(ND):
                    _mm(nc, psw[:], wkc[:, j:j + 1], keysTt[j][:],
                        j == 0, j == ND - 1)
                simw = small.tile([1, K], F32, tag="simw")
                nc.vector.tensor_scalar(simw[:], psw[:], rw[:], None, ALU.mult)
                nc.vector.tensor_tensor(simw[:], simw[:], rk_r[:], ALU.mult)
                # writer softmax*50, hard one-hot
                bs = small.tile([1, 1], F32, tag="bs")
                nc.vector.reduce_max(bs[:], simw[:], axis=AX)
                nb = small.tile([1, 1], F32, tag="nb")
                nc.vector.tensor_scalar(nb[:], bs[:], -50.0, None, ALU.mult)
                ew = small.tile([1, K], F32, tag="ew")
                nc.scalar.activation(ew[:], simw[:], AF.Exp, scale=50.0,
                                     bias=nb[:])
                wsum = small.tile([1, 1], F32, tag="wsum")
                nc.vector.reduce_sum(wsum[:], ew[:], axis=AX)
                nc.vector.reciprocal(wsum[:], wsum[:])
                soft = small.tile([1, K], F32, tag="soft")
                nc.vector.tensor_scalar(soft[:], ew[:], wsum[:], None,
                                        ALU.mult)
                smax = small.tile([1, 1], F32, tag="smax")
                nc.vector.reduce_max(smax[:], soft[:], axis=AX)
                hard = small.tile([1, K], F32, tag="hard")
                nc.vector.tensor_scalar(hard[:], soft[:], smax[:], None,
                                        ALU.is_ge)
                ww = small.tile([1, K], F32, tag="ww")
                nc.vector.tensor_tensor(ww[:], hard[:], soft[:], ALU.subtract)
                nc.vector.tensor_tensor(ww[:], ww[:], soft[:], ALU.add)
                # ws = sigmoid(q_win @ Wws + bws) via exp
                pss = ps.tile([1, 1], F32, tag="ps")
                for j in range(ND):
                    _mm(nc, pss[:], qwc[:, j:j + 1], wws_c[:, j:j + 1],
                        j == 0, j == ND - 1)
                wspre = small.tile([1, 1], F32, tag="wspre")
                nc.vector.scalar_tensor_tensor(wspre[:], pss[:], 1.0 / D,
                                               bws_s[:], ALU.mult, ALU.add)
                wss = small.tile([1, 1], F32, tag="wss")
                nc.scalar.activation(wss[:], wspre[:], AF.Exp, scale=-1.0)
                nc.vector.tensor_scalar(wss[:], wss[:], 1.0, None, ALU.add)
                nc.vector.reciprocal(wss[:], wss[:])
                # eff, cols
                eff = small.tile([1, K], F32, tag="eff")
                nc.vector.tensor_scalar(eff[:], ww[:], wss[:], 0.5,
                                        ALU.mult, ALU.mult)
                pec = ps.tile([64, 1], F32, tag="ps")
                nc.tensor.transpose(pec[:], eff[:], ident[0:1, 0:1])
                effc = small.tile([64, 1], F32, tag="effc")
                nc.vector.tensor_copy(effc[:], pec[:])
                omec = small.tile([64, 1], F32, tag="omec")
                nc.vector.tensor_scalar(omec[:], effc[:], -1.0, 1.0,
                                        ALU.mult, ALU.add)
                # keys_new
                pkb = ps.tile([64, D], F32, tag="ps")
                _mm(nc, pkb[:], ones[0:1, 0:64], wk_r[:], True, True)
                t1 = cp.tile([64, D], F32, tag="t1C")
                nc.vector.tensor_scalar(t1[:], keysN[:], omec[:], None,
                                        ALU.mult)
                kraw = cp.tile([64, D], F32, tag="krawC")
                nc.vector.scalar_tensor_tensor(kraw[:], pkb[:], effc[:],
                                               t1[:], ALU.mult, ALU.add)
                krssq = small.tile([64, 1], F32, tag="krssq")
                kscr = cp.tile([64, D], F32, tag="kscrC")
                nc.vector.scalar_tensor_tensor(kscr[:], kraw[:], 1.0, kraw[:],
                                               ALU.mult, ALU.mult,
                                               accum_out=krssq[:])
                krn = small.tile([64, 1], F32, tag="krn")
                ln_exp_scale(krn[:], krssq[:], 1.0, 0.0, 0.5)
                nc.vector.tensor_scalar(krn[:], krn[:], 1e-6, None, ALU.add)
                nc.vector.reciprocal(krn[:], krn[:])
                knew = cp.tile([64, D], F32, tag="knewC")
                nc.vector.tensor_scalar(knew[:], kraw[:], krn[:], None,
                                        ALU.mult)
                nc.sync.dma_start(kn_d.ap()[b], knew[:])
                # vals_new
                pvb = ps.tile([64, D], F32, tag="ps")
                _mm(nc, pvb[:], ones[0:1, 0:64], wv_r[:], True, True)
                t1v = cp.tile([64, D], F32, tag="t1vC")
                nc.vector.tensor_scalar(t1v[:], valsN[:], omec[:], None,
                                        ALU.mult)
                vnew = cp.tile([64, D], F32, tag="vnewC")
                nc.vector.scalar_tensor_tensor(vnew[:], pvb[:], effc[:],
                                               t1v[:], ALU.mult, ALU.add)
                nc.sync.dma_start(vn_d.ap()[b], vnew[:])
                # age_new, strength_new
                omw = small.tile([1, K], F32, tag="omw")
                nc.vector.tensor_scalar(omw[:], ww[:], -1.0, 1.0,
                                        ALU.mult, ALU.add)
                anew = small.tile([1, K], F32, tag="anew")
                nc.vector.scalar_tensor_tensor(anew[:], age_r[:], 1.0, omw[:],
                                               ALU.add, ALU.mult)
                nc.sync.dma_start(an_d.ap()[b:b + 1, :], anew[:])
                s995 = small.tile([1, K], F32, tag="s995")
                nc.vector.tensor_scalar(s995[:], str_r[:], 0.995, None,
                                        ALU.mult)
                oms = small.tile([1, K], F32, tag="oms")
                nc.vector.tensor_scalar(oms[:], s995[:], -1.0, 1.0,
                                        ALU.mult, ALU.add)
                wws2 = small.tile([1, K], F32, tag="wws2")
                nc.vector.tensor_scalar(wws2[:], eff[:], 2.0, None, ALU.mult)
                nc.vector.tensor_tensor(wws2[:], wws2[:], oms[:], ALU.mult)
                snew = small.tile([1, K], F32, tag="snew")
                nc.vector.tensor_tensor(snew[:], s995[:], wws2[:], ALU.add)
                nc.vector.tensor_scalar(snew[:], snew[:], 0.001, 1.0,
                                        ALU.max, ALU.min)
                nc.sync.dma_start(sn_d.ap()[b:b + 1, :], snew[:])
                # gate MLP
                pg1 = ps.tile([1, D], F32, tag="ps")
                for j in range(ND):
                    _mm(nc, pg1[:], qwc[:, j:j + 1], wg1[j][:],
                        j == 0, j == ND - 1)
                g1 = small.tile([1, D], F32, tag="g1")
                nc.vector.scalar_tensor_tensor(g1[:], pg1[:], 1.0 / D,
                                               bg1_r[:], ALU.mult, ALU.add)
                nov = small.tile([1, 1], F32, tag="nov")
                nc.vector.tensor_scalar(nov[:], bs[:], -1.0, 1.0,
                                        ALU.mult, ALU.add)
                wsnov_r = small.tile([1, 2], F32, tag="wsnovr")
                nc.vector.tensor_copy(wsnov_r[:, 0:1], wss[:])
                nc.vector.tensor_copy(wsnov_r[:, 1:2], nov[:])
                pwsn = ps.tile([2, 1], F32, tag="ps")
                nc.tensor.transpose(pwsn[:], wsnov_r[:], ident[0:1, 0:1])
                wsnov = small.tile([2, 1], F32R, tag="wsnov")
                nc.vector.tensor_copy(wsnov[:], pwsn[:])
                pg2 = ps.tile([1, D], F32, tag="ps")
                _mm(nc, pg2[:], wsnov[:], wg1t_r[:], True, True)
                nc.vector.tensor_tensor(g1[:], g1[:], pg2[:], ALU.add)
                # silu(g1) = g1 * sigmoid(g1) via exp
                eg = small.tile([1, D], F32, tag="eg")
                nc.scalar.activation(eg[:], g1[:], AF.Exp, scale=-1.0)
                nc.vector.tensor_scalar(eg[:], eg[:], 1.0, None, ALU.add)
                nc.vector.reciprocal(eg[:], eg[:])
                sg1 = small.tile([1, D], F32, tag="sg1")
                nc.vector.tensor_tensor(sg1[:], g1[:], eg[:], ALU.mult)
                psgc = ps.tile([128, ND], F32, tag="ps")
                for j in range(ND):
                    nc.tensor.transpose(psgc[:, j:j + 1],
                                        sg1[0:1, 128 * j:128 * (j + 1)],
                                        ident[0:1, 0:1])
                sgc = small.tile([128, ND], F32R, tag="sgc")
                nc.vector.tensor_copy(sgc[:], psgc[:])
                pgp = ps.tile([1, 1], F32, tag="ps")
                for j in range(ND):
                    _mm(nc, pgp[:], sgc[:, j:j + 1], wg2_c[:, j:j + 1],
                        j == 0, j == ND - 1)
                gpre = small.tile([1, 1], F32, tag="gpre")
                nc.vector.tensor_tensor(gpre[:], pgp[:], bg2_s[:], ALU.add)
                gat = small.tile([1, 1], F32, tag="gat")
                nc.scalar.activation(gat[:], gpre[:], AF.Exp, scale=-1.0)
                nc.vector.tensor_scalar(gat[:], gat[:], 1.0, None, ALU.add)
                nc.vector.reciprocal(gat[:], gat[:])
                nc.sync.dma_start(gate_d.ap()[b:b + 1, :], gat[:])
                # out = h2 + gate*read  (broadcast over t), then transpose out
                gr = small.tile([1, D], F32, tag="gr")
                nc.vector.tensor_scalar(gr[:], read_r[:], gat[:], None,
                                        ALU.mult)
                pgr = ps.tile([128, ND], F32, tag="ps")
                for j in range(ND):
                    nc.tensor.transpose(pgr[:, j:j + 1],
                                        gr[0:1, 128 * j:128 * (j + 1)],
                                        ident[0:1, 0:1])
                grc = small.tile([128, ND], F32, tag="grc")
                nc.vector.tensor_copy(grc[:], pgr[:])
                for j in range(ND):
                    nc.vector.tensor_scalar(h2[j][:], h2[j][:],
                                            grc[:, j:j + 1], None, ALU.add)
                for i in range(NT):
                    pot = ps.tile([128, D], F32, tag="ps")
                    for j in range(ND):
                        nc.tensor.transpose(pot[:, 128 * j:128 * (j + 1)],
                                            h2[j][:, 128 * i:128 * (i + 1)],
                                            ident[:])
                    ot = cp.tile([128, D], F32, tag="outN")
                    nc.scalar.copy(ot[:], pot[:])
                    nc.sync.dma_start(out_d.ap()[b, 128 * i:128 * (i + 1), :],
                                      ot[:])

    nc.compile()
    return nc


_NC_CACHE = {}


def _get_nc():
    if "nc" not in _NC_CACHE:
        _NC_CACHE["nc"] = _build()
    return _NC_CACHE["nc"]


def _get_runner():
    """Build the sharded jitted executable once (mirrors run_bass_via_pjrt)."""
    if "runner" in _NC_CACHE:
        return _NC_CACHE["runner"]
    import jax
    import concourse.mybir as mybir_
    from concourse import bass2jax
    from jax.experimental.shard_map import shard_map
    from jax.sharding import Mesh, PartitionSpec

    nc = _get_nc()
    bass2jax.install_neuronx_cc_hook()
    partition_name = (nc.partition_id_tensor.name
                      if nc.partition_id_tensor else None)
    in_names, out_names, out_avals, zero_shapes = [], [], [], []
    for alloc in nc.m.functions[0].allocations:
        if not isinstance(alloc, mybir_.MemoryLocationSet):
            continue
        name = alloc.memorylocations[0].name
        if alloc.kind == "ExternalInput":
            if name != partition_name:
                in_names.append(name)
        elif alloc.kind == "ExternalOutput":
            out_names.append(name)
            shape = tuple(alloc.tensor_shape)
            dtype = mybir_.dt.np(alloc.dtype)
            out_avals.append(jax.core.ShapedArray(shape, dtype))
            zero_shapes.append((shape, dtype))
    n_params = len(in_names)
    all_names = list(in_names) + list(out_names)
    if partition_name is not None:
        all_names.append(partition_name)
    donate = tuple(range(n_params, n_params + len(out_names)))

    def _body(*args):
        operands = list(args)
        if partition_name is not None:
            operands.append(bass2jax.partition_id_tensor())
        outs = bass2jax._bass_exec_p.bind(
            *operands,
            out_avals=tuple(out_avals),
            in_names=tuple(all_names),
            out_names=tuple(out_names),
            lowering_input_output_aliases=(),
            sim_require_finite=True,
            sim_require_nnan=True,
            nc=nc,
        )
        return tuple(outs)

    devices = jax.devices()[:NCORES]
    mesh = Mesh(np.asarray(devices), ("core",))
    n_out = len(out_names)
    in_specs = (PartitionSpec("core"),) * (n_params + n_out)
    out_specs = (PartitionSpec("core"),) * n_out
    fn = jax.jit(
        shard_map(_body, mesh=mesh, in_specs=in_specs, out_specs=out_specs,
                  check_rep=False),
        donate_argnums=donate, keep_unused=True)
    runner = {"fn": fn, "in_names": in_names, "out_names": out_names,
              "zero_shapes": zero_shapes, "mesh": mesh}
    _NC_CACHE["runner"] = runner
    return runner


def _concat_inputs(in_maps, runner):
    return [np.concatenate([np.asarray(m[n]) for m in in_maps], axis=0)
            for n in runner["in_names"]]


def _make_zeros(runner):
    return [np.zeros((NCORES * s[0], *s[1:]), d)
            for s, d in runner["zero_shapes"]]


def _split_outs(out_arrs, runner):
    res = [{} for _ in range(NCORES)]
    for i, n in enumerate(runner["out_names"]):
        arr = np.asarray(out_arrs[i])
        per = arr.shape[0] // NCORES
        for c in range(NCORES):
            res[c][n] = arr[c * per:(c + 1) * per]
    return res


def _swap_matrix():
    # psw = swapm.T @ q swaps the re/im 32-blocks within each 64-partition
    # head block (deinterleaved rope layout)
    sw = np.zeros((128, 128), np.float32)
    for a in (0, 64):
        for i in range(32):
            sw[a + 32 + i, a + i] = 1.0
            sw[a + i, a + 32 + i] = 1.0
    return sw


def _host_prep(inputs):
    f = lambda n: np.ascontiguousarray(np.asarray(inputs[n], np.float32))
    x = f("x")
    keys = f("epi_keys")
    vals = f("epi_vals")
    age = f("epi_age")
    strength = f("epi_strength")
    pos = np.asarray(inputs["pos_idx"]).astype(np.float64)

    # deinterleave perm per head: evens then odds
    ph = np.concatenate([np.arange(0, DH, 2), np.arange(1, DH, 2)])
    perm = np.concatenate([h * DH + ph for h in range(H)])
    wq_p = f("Wq")[:, perm]
    wk_p = f("Wk")[:, perm]
    bq_p = f("bq")[perm]
    bk_p = f("bk")[perm]

    freqs = 1.0 / (10000.0 ** (np.arange(0, DH, 2, dtype=np.float64) / DH))
    ang = pos[None, :] * freqs[:, None]          # [32, T]
    cos32 = np.cos(ang).astype(np.float32)
    sin32 = np.sin(ang).astype(np.float32)
    cosT = np.empty((128, T), np.float32)
    sinT = np.empty((128, T), np.float32)
    for blk in range(2):
        o = blk * 64
        cosT[o:o + 32] = cos32
        cosT[o + 32:o + 64] = cos32
        sinT[o:o + 32] = -sin32
        sinT[o + 32:o + 64] = sin32

    mask = np.triu(np.ones((T, T), np.float32))  # mask[s,t] = 1 if s<=t

    common = {
        "Wq": wq_p, "Wk": wk_p, "Wv": f("Wv"), "Wo": f("Wo"),
        "bq": bq_p, "bk": bk_p,
        "bv_bcast": np.tile(f("bv")[None, :], (128, 1)),
        "bo": f("bo"), "W1": f("W1"), "W2": f("W2"),
        "Wwk": f("Wwk"), "Wwv": f("Wwv"),
        "bwk": f("bwk"), "bwv": f("bwv"),
        "Wws": f("Wws").reshape(D), "bws": f("bws").reshape(1, 1),
        "Wg1": f("Wg1"), "bg1": f("bg1").reshape(1, D),
        "Wg2": f("Wg2").reshape(D), "bg2": f("bg2").reshape(1, 1),
        "rms1": f("rms1"), "rms_kv": f("rms_kv"), "rms2": f("rms2"),
        "rms_read": f("rms_read").reshape(1, D),
        "rope_cos": cosT, "rope_sin": sinT, "mask": mask,
        "ident": np.eye(128, dtype=np.float32),
        "ones": np.ones((128, 128), np.float32),
        "swapm": _swap_matrix(),
    }
    keysT = np.ascontiguousarray(keys.transpose(0, 2, 1))
    in_maps = []
    for c in range(NCORES):
        s = slice(c * BL, (c + 1) * BL)
        m = dict(common)
        m.update({"x": x[s], "keys": keys[s], "keysT": keysT[s],
                  "vals": vals[s], "age": age[s], "strength": strength[s]})
        in_maps.append(m)
    return in_maps


def kernel(**inputs):
    runner = _get_runner()
    in_maps = _host_prep(inputs)
    out_arrs = runner["fn"](*_concat_inputs(in_maps, runner),
                            *_make_zeros(runner))
    res = _split_outs(out_arrs, runner)
    cat = lambda n: np.concatenate([res[c][n] for c in range(NCORES)], axis=0)
    return (cat("out"), cat("keys_new"), cat("vals_new"), cat("age_new"),
            cat("strength_new"), cat("gate"))


def timed_run(inputs, iters=8):
    """Device-resident timing: returns (per_call_seconds_list, results)."""
    import time as _time
    import jax
    runner = _get_runner()
    in_maps = _host_prep(inputs)
    dev_ins = [jax.device_put(a) for a in _concat_inputs(in_maps, runner)]
    fn = runner["fn"]
    zero_sets = [[jax.device_put(z) for z in _make_zeros(runner)]
                 for _ in range(iters + 1)]
    for zs in zero_sets:
        jax.block_until_ready(zs)
    # warmup
    out = fn(*dev_ins, *zero_sets[0])
    jax.block_until_ready(out)
    times = []
    for it in range(iters):
        t0 = _time.perf_counter()
        out = fn(*dev_ins, *zero_sets[it + 1])
        jax.block_until_ready(out)
        times.append(_time.perf_counter() - t0)
    return times, out


# revision 51
# speedup vs baseline: 1.2481x; 1.2481x over previous
import numpy as np
from contextlib import ExitStack

import concourse.bass as bass
import concourse.tile as tile
import concourse.mybir as mybir
from concourse import bacc
from concourse.bass_utils import run_bass_kernel_spmd

B, T, D, H, DH, K = 32, 512, 512, 8, 64, 64
NCORES = 8
BL = B // NCORES          # 4 batches per core
NT = T // 128             # 4 t-tiles
ND = D // 128             # 4 d-tiles
NJ = 16                   # FFN hidden tiles (2048/128)
F32 = mybir.dt.float32
F32R = mybir.dt.float32r
AF = mybir.ActivationFunctionType
ALU = mybir.AluOpType
AX = mybir.AxisListType.X

MM_DT = mybir.dt.float32r  # fast fp32r matmuls (4x PE throughput at N>=256)


def _mm(nc, out, lhsT, rhs, start, stop):
    n = out.shape[-1]
    dt_ = MM_DT if (MM_DT is not None and n >= 256) else F32
    lhsT = lhsT.bitcast(dt_)
    rhs = rhs.bitcast(dt_)
    nc.tensor.matmul(out, lhsT, rhs, start=start, stop=stop)


def _patch_act_tables():
    # The act-table-load pass picks the FIRST set containing each function,
    # so exp->exp_and_others and ln->natural_log ping-pong (2.7us per load).
    # Hide exp/ln from every set except natural_log_exp_and_others so both
    # resolve to the one set that holds them jointly. Safe: ids stay the
    # act_info.json indices; we only narrow the pass's choice.
    from concourse import hw_specs
    orig = hw_specs.get_activation_tables
    if getattr(bacc, "_act_tables_patched", False):
        return
    def patched(arch):
        out = {}
        for name, fns in orig(arch).items():
            fns = set(fns)
            if name != "natural_log_exp_and_others":
                fns.discard(AF.Exp)
                fns.discard(AF.Ln)
            out[name] = fns
        return out
    bacc.get_activation_tables = patched
    bacc._act_tables_patched = True


def _build():
    _patch_act_tables()
    nc = bacc.Bacc("TRN2", target_bir_lowering=False, debug=False,
                   num_devices=NCORES)
    dt = nc.dram_tensor
    # inputs (per core shapes)
    x_d = dt("x", [BL, T, D], F32, kind="ExternalInput")
    keys_d = dt("keys", [BL, K, D], F32, kind="ExternalInput")
    keysT_d = dt("keysT", [BL, D, K], F32R, kind="ExternalInput")
    vals_d = dt("vals", [BL, K, D], F32R, kind="ExternalInput")
    age_d = dt("age", [BL, K], F32, kind="ExternalInput")
    str_d = dt("strength", [BL, K], F32, kind="ExternalInput")
    wq_d = dt("Wq", [D, D], F32R, kind="ExternalInput")
    wk_d = dt("Wk", [D, D], F32R, kind="ExternalInput")
    wv_d = dt("Wv", [D, D], F32R, kind="ExternalInput")
    wo_d = dt("Wo", [D, D], F32R, kind="ExternalInput")
    bq_d = dt("bq", [D], F32, kind="ExternalInput")
    bk_d = dt("bk", [D], F32, kind="ExternalInput")
    bvb_d = dt("bv_bcast", [128, D], F32, kind="ExternalInput")
    rms2b_d = dt("rms2_bcast", [128, D], F32, kind="ExternalInput")
    bo_d = dt("bo", [D], F32R, kind="ExternalInput")
    w1_d = dt("W1", [D, 8 * D], F32R, kind="ExternalInput")
    w2_d = dt("W2", [4 * D, D], F32R, kind="ExternalInput")
    wwk_d = dt("Wwk", [D, D], F32R, kind="ExternalInput")
    wwv_d = dt("Wwv", [D, D], F32R, kind="ExternalInput")
    bwk_d = dt("bwk", [D], F32, kind="ExternalInput")
    bwv_d = dt("bwv", [D], F32, kind="ExternalInput")
    wws_d = dt("Wws", [D], F32R, kind="ExternalInput")
    bws_d = dt("bws", [1, 1], F32, kind="ExternalInput")
    wg1_d = dt("Wg1", [D + 2, D], F32R, kind="ExternalInput")
    bg1_d = dt("bg1", [1, D], F32, kind="ExternalInput")
    wg2_d = dt("Wg2", [D], F32R, kind="ExternalInput")
    bg2_d = dt("bg2", [1, 1], F32, kind="ExternalInput")
    rms1_d = dt("rms1", [D], F32, kind="ExternalInput")
    rmskv_d = dt("rms_kv", [D], F32, kind="ExternalInput")
    rms2_d = dt("rms2", [D], F32, kind="ExternalInput")
    rmsrd_d = dt("rms_read", [1, D], F32, kind="ExternalInput")
    cos_d = dt("rope_cos", [128, T], F32, kind="ExternalInput")
    sin_d = dt("rope_sin", [128, T], F32, kind="ExternalInput")
    mask_d = dt("mask", [T, T], F32, kind="ExternalInput")
    id_d = dt("ident", [128, 128], F32, kind="ExternalInput")
    ones_d = dt("ones", [128, 128], F32R, kind="ExternalInput")
    swap_d = dt("swapm", [128, 128], F32R, kind="ExternalInput")
    # outputs
    out_d = dt("out", [BL, T, D], F32, kind="ExternalOutput")
    kn_d = dt("keys_new", [BL, K, D], F32, kind="ExternalOutput")
    vn_d = dt("vals_new", [BL, K, D], F32, kind="ExternalOutput")
    an_d = dt("age_new", [BL, K], F32, kind="ExternalOutput")
    sn_d = dt("strength_new", [BL, K], F32, kind="ExternalOutput")
    gate_d = dt("gate", [BL, 1], F32, kind="ExternalOutput")
    h1s_d = dt("h1scratch", [BL, T, D], F32)

    def colview(d):  # [512] dram -> [128, 4] col tile view
        return d.ap().rearrange("(j p) -> p j", p=128)

    with tile.TileContext(nc) as tc, ExitStack() as octx, \
            nc.allow_low_precision(reason="fp32r matmul operands"):
        cst = octx.enter_context(tc.tile_pool(name="cst", bufs=1))
        ps = octx.enter_context(tc.tile_pool(name="ps", bufs=7, space="PSUM"))

        ident = cst.tile([128, 128], F32, tag="ident")
        nc.sync.dma_start(ident[:], id_d.ap())
        ones = cst.tile([128, 128], F32R, tag="ones")
        nc.sync.dma_start(ones[:], ones_d.ap())
        swapm = cst.tile([128, 128], F32R, tag="swapm")
        nc.sync.dma_start(swapm[:], swap_d.ap())
        cosT = cst.tile([128, T], F32, tag="cos")
        nc.sync.dma_start(cosT[:], cos_d.ap())
        sinT = cst.tile([128, T], F32, tag="sin")
        nc.sync.dma_start(sinT[:], sin_d.ap())
        bq_c = cst.tile([128, ND], F32, tag="bqc")
        nc.sync.dma_start(bq_c[:], colview(bq_d))
        bk_c = cst.tile([128, ND], F32, tag="bkc")
        nc.sync.dma_start(bk_c[:], colview(bk_d))
        bo_r = cst.tile([1, D], F32R, tag="bor")
        nc.sync.dma_start(bo_r[:], bo_d.ap().rearrange("(o d) -> o d", o=1))
        rms2b = cst.tile([128, D], F32, tag="rms2b")
        nc.sync.dma_start(rms2b[:], rms2b_d.ap())
        rms1_c = cst.tile([128, ND], F32, tag="rms1c")
        nc.sync.dma_start(rms1_c[:], colview(rms1_d))
        rmskv_c = cst.tile([128, ND], F32, tag="rmskvc")
        nc.sync.dma_start(rmskv_c[:], colview(rmskv_d))
        wws_c = cst.tile([128, ND], F32R, tag="wwsc")
        nc.sync.dma_start(wws_c[:], colview(wws_d))
        wg2_c = cst.tile([128, ND], F32R, tag="wg2c")
        nc.sync.dma_start(wg2_c[:], colview(wg2_d))
        rowpack = cst.tile([128, D], F32, tag="rowpack")
        rmsrd_r = rowpack[0:1, :]
        nc.sync.dma_start(rmsrd_r, rmsrd_d.ap())
        bg1_r = rowpack[32:33, :]
        nc.sync.dma_start(bg1_r, bg1_d.ap())
        bwk_r = rowpack[64:65, :]
        nc.sync.dma_start(bwk_r, bwk_d.ap().rearrange("(o d) -> o d", o=1))
        bwv_r = rowpack[96:97, :]
        nc.sync.dma_start(bwv_r, bwv_d.ap().rearrange("(o d) -> o d", o=1))
        bws_s = cst.tile([1, 1], F32, tag="bwss")
        nc.sync.dma_start(bws_s[:], bws_d.ap())
        bg2_s = cst.tile([1, 1], F32, tag="bg2s")
        nc.sync.dma_start(bg2_s[:], bg2_d.ap())
        bvb = cst.tile([128, D], F32, tag="bvb")
        nc.sync.dma_start(bvb[:], bvb_d.ap())
        wg1t_r = cst.tile([2, D], F32R, tag="wg1t")
        nc.sync.dma_start(wg1t_r[:], wg1_d.ap()[D:D + 2, :])
        eps_c = cst.tile([128, 1], F32, tag="epsc")
        nc.vector.memset(eps_c[:], 1e-6)



        def ln_exp_scale(out_ap, in_ap, ln_scale, ln_bias, exp_scale):
            # out = exp(exp_scale * ln(ln_scale*in + ln_bias))
            t = small.tile(list(in_ap.shape), F32, tag="lnexp", name="lnexp_t")
            bias = eps_c[0:in_ap.shape[0], 0:1] if ln_bias else 0.0
            nc.scalar.activation(t[:], in_ap, AF.Ln, scale=ln_scale,
                                 bias=bias)
            nc.scalar.activation(out_ap, t[:], AF.Exp, scale=exp_scale)

        # ---------------- Phase A: attention -> h1T ----------------
        with ExitStack() as actx:
            wpool = actx.enter_context(tc.tile_pool(name="wA", bufs=1))
            apool = actx.enter_context(tc.tile_pool(name="actA", bufs=4))
            epool = actx.enter_context(tc.tile_pool(name="eA", bufs=5))
            spool = actx.enter_context(tc.tile_pool(name="sA", bufs=4))
            small = actx.enter_context(tc.tile_pool(name="smA", bufs=4))

            wq = [wpool.tile([128, D], F32R, tag=f"wq{i}", name=f"wq{i}") for i in range(ND)]
            wk = [wpool.tile([128, D], F32R, tag=f"wk{i}", name=f"wk{i}") for i in range(ND)]
            wv = [wpool.tile([128, D], F32R, tag=f"wv{i}", name=f"wv{i}") for i in range(ND)]
            wo = [wpool.tile([128, D], F32R, tag=f"wo{i}", name=f"wo{i}") for i in range(ND)]
            for i in range(ND):
                nc.sync.dma_start(wq[i][:], wq_d.ap()[128 * i:128 * (i + 1), :])
                nc.sync.dma_start(wk[i][:], wk_d.ap()[128 * i:128 * (i + 1), :])
                nc.sync.dma_start(wv[i][:], wv_d.ap()[128 * i:128 * (i + 1), :])
                nc.sync.dma_start(wo[i][:], wo_d.ap()[128 * i:128 * (i + 1), :])
            maskd = wpool.tile([128, 128], F32, tag="maskd")
            nc.sync.dma_start(maskd[:], mask_d.ap()[0:128, 0:128])

            for b in range(NB):
                # load x, rms stats
                xt, rcol = [], []
                for i in range(NT):
                    t = apool.tile([128, D], F32, tag="x", bufs=8)
                    nc.sync.dma_start(t[:], x_d.ap()[b, 128 * i:128 * (i + 1), :])
                    xt.append(t)
                for i in range(NT):
                    sq = spool.tile([128, D], F32, tag="sq", bufs=2)
                    rc = small.tile([128, 1], F32, tag="rc")
                    nc.vector.scalar_tensor_tensor(
                        sq[:], xt[i][:], 1.0, xt[i][:], ALU.mult, ALU.mult,
                        accum_out=rc[:])
                    r = small.tile([128, 1], F32, tag="rr")
                    ln_exp_scale(r[:], rc[:], 1.0 / D, 1e-6, -0.5)
                    rcol.append(r)
                # xn in own tiles, then transpose -> xnT
                xn = []
                for i in range(NT):
                    xni = spool.tile([128, D], F32, tag="xn", name=f"xn{i}")
                    nc.vector.tensor_scalar(xni[:], xt[i][:], rcol[i][:],
                                            None, ALU.mult)
                    xn.append(xni)
                nxT, nkvT = [], []
                for j in range(ND):
                    pt = ps.tile([128, T], F32, tag="ps")
                    for i in range(NT):
                        nc.tensor.transpose(
                            pt[:, 128 * i:128 * (i + 1)],
                            xn[i][:, 128 * j:128 * (j + 1)], ident[:])
                    t = apool.tile([128, T], F32R, tag="nxT")
                    nc.scalar.activation(t[:], pt[:], AF.Identity,
                                         scale=rms1_c[:, j:j + 1])
                    nxT.append(t)
                    t2 = apool.tile([128, T], F32R, tag="nkvT")
                    nc.vector.tensor_scalar(t2[:], pt[:],
                                            rmskv_c[:, j:j + 1], None, ALU.mult)
                    nkvT.append(t2)
                # projections
                qT, kT, vN = [], [], []
                for j in range(ND):
                    pq = ps.tile([128, T], F32, tag="ps")
                    for i in range(ND):
                        _mm(nc, pq[:], wq[i][:, 128 * j:128 * (j + 1)],
                            nxT[i][:], i == 0, i == ND - 1)
                    t = apool.tile([128, T], F32R, tag="qT", bufs=8)
                    nc.scalar.activation(t[:], pq[:], AF.Identity,
                                         bias=bq_c[:, j:j + 1])
                    qT.append(t)
                    pk = ps.tile([128, T], F32, tag="ps")
                    for i in range(ND):
                        _mm(nc, pk[:], wk[i][:, 128 * j:128 * (j + 1)],
                            nkvT[i][:], i == 0, i == ND - 1)
                    t = apool.tile([128, T], F32R, tag="kT", bufs=8)
                    nc.scalar.activation(t[:], pk[:], AF.Identity,
                                         bias=bk_c[:, j:j + 1])
                    kT.append(t)
                for i in range(NT):
                    pv = ps.tile([128, D], F32, tag="ps")
                    for kk in range(ND):
                        _mm(nc, pv[:], nkvT[kk][:, 128 * i:128 * (i + 1)],
                            wv[kk][:], kk == 0, kk == ND - 1)
                    t = apool.tile([128, D], F32R, tag="vN", bufs=6)
                    nc.vector.tensor_tensor(t[:], pv[:], bvb[:], ALU.add)
                    vN.append(t)
                # rope on qT, kT (in place): rot = q*cos + swap(q)*sin
                for lst in (qT, kT):
                    for j in range(ND):
                        psw = ps.tile([128, T], F32, tag="ps")
                        _mm(nc, psw[:], swapm[:], lst[j][:], True, True)
                        qc = spool.tile([128, T], F32, tag="ropeC", bufs=2)
                        nc.vector.tensor_tensor(qc[:], lst[j][:], cosT[:],
                                                ALU.mult)
                        tm = spool.tile([128, T], F32, tag="ropeS", bufs=2)
                        nc.vector.tensor_tensor(tm[:], psw[:], sinT[:],
                                                ALU.mult)
                        nc.vector.tensor_tensor(lst[j][:], qc[:], tm[:],
                                                ALU.add)
                # attention heads
                oT = [apool.tile([128, T], F32R, tag="oT", name=f"oT{j}", bufs=6) for j in range(ND)]
                for h in range(H):
                    jt, pr = h // 2, (h % 2) * 64
                    pcs = ps.tile([1, T], F32, tag="ps")
                    po = ps.tile([64, T], F32, tag="ps")
                    for i in range(NT):
                        ni = T - 128 * i
                        pss = ps.tile([128, T], F32, tag="ps")
                        _mm(nc, pss[:, 0:ni],
                            kT[jt][pr:pr + 64, 128 * i:128 * (i + 1)],
                            qT[jt][pr:pr + 64, 128 * i:T], True, True)
                        e = epool.tile([128, T], F32R, tag="E")
                        nc.scalar.activation(e[:, 0:ni], pss[:, 0:ni], AF.Exp,
                                             scale=1.0 / 8.0)
                        nc.vector.tensor_tensor(e[:, 0:128], e[:, 0:128],
                                                maskd[:], ALU.mult)
                        _mm(nc, pcs[0:1, 128 * i:T], ones[:, 0:1], e[:, 0:ni],
                            i == 0, i == NT - 1)
                        _mm(nc, po[0:64, 128 * i:T],
                            vN[i][:, 64 * h:64 * (h + 1)], e[:, 0:ni],
                            i == 0, i == NT - 1)
                    rr = small.tile([1, T], F32R, tag="csr")
                    nc.vector.reciprocal(rr[:], pcs[:])
                    pbc = ps.tile([64, T], F32, tag="ps")
                    _mm(nc, pbc[:], ones[0:1, 0:64], rr[:], True, True)
                    bcs = spool.tile([64, T], F32, tag="bcs", bufs=3)
                    nc.scalar.copy(bcs[:], pbc[:])
                    nc.vector.tensor_tensor(oT[jt][pr:pr + 64, :], po[:],
                                            bcs[:], ALU.mult)
                # Wo projection (natural) + bo (K=1 matmul) + residual
                for i in range(NT):
                    ph = ps.tile([128, D], F32, tag="ps")
                    for kk in range(ND):
                        _mm(nc, ph[:], oT[kk][:, 128 * i:128 * (i + 1)],
                            wo[kk][:], kk == 0, False)
                    _mm(nc, ph[:], ones[0:1, :], bo_r[:], False, True)
                    h1o = spool.tile([128, D], F32, tag="h1o", bufs=3)
                    nc.vector.tensor_tensor(h1o[:], ph[:], xt[i][:], ALU.add)
                    nc.sync.dma_start(h1s_d.ap()[b, 128 * i:128 * (i + 1), :],
                                      h1o[:])

        # ---------------- Phase B: FFN (h2 accumulated into h1T) --------
        with ExitStack() as bctx:
            w1p = bctx.enter_context(tc.tile_pool(name="w1p", bufs=2))
            w2p = bctx.enter_context(tc.tile_pool(name="w2p", bufs=8))
            hnp = bctx.enter_context(tc.tile_pool(name="hnp", bufs=BL * ND))
            bsc = bctx.enter_context(tc.tile_pool(name="bsc", bufs=3))
            apb = bctx.enter_context(tc.tile_pool(name="apb", bufs=12))
            small = bctx.enter_context(tc.tile_pool(name="smB", bufs=6))

            h1nT = []
            for b in range(BL):
                # rms over d (partition dim) via ones-matmul of squares
                pssq = ps.tile([1, T], F32, tag="ps")
                for j in range(ND):
                    sq = bsc.tile([128, T], F32R, tag="sqB")
                    nc.vector.tensor_tensor(sq[:], h1T[b][j][:], h1T[b][j][:],
                                            ALU.mult)
                    _mm(nc, pssq[:], ones[:, 0:1], sq[:], j == 0, j == ND - 1)
                rrow = small.tile([1, T], F32R, tag="rrowB")
                ln_exp_scale(rrow[:], pssq[:], 1.0 / D, 1e-6, -0.5)
                prb = ps.tile([128, T], F32, tag="ps")
                _mm(nc, prb[:], ones[0:1, :], rrow[:], True, True)
                rb = bsc.tile([128, T], F32, tag="rbB")
                nc.scalar.copy(rb[:], prb[:])
                row = []
                for j in range(ND):
                    t = hnp.tile([128, T], F32R, tag="h1n")
                    nc.vector.scalar_tensor_tensor(
                        t[:], h1T[b][j][:], rms2_c[:, j:j + 1], rb[:],
                        ALU.mult, ALU.mult)
                    row.append(t)
                h1nT.append(row)

            for jg in range(4):
                js = [4 * jg + a for a in range(4)]
                w1g, w1v, w2t = {}, {}, {}
                for j in js:
                    tg = w1p.tile([128, D], F32R, tag="w1g")
                    for kk in range(ND):
                        nc.sync.dma_start(
                            tg[:, 128 * kk:128 * (kk + 1)],
                            w1_d.ap()[128 * kk:128 * (kk + 1),
                                      128 * j:128 * (j + 1)])
                    w1g[j] = tg
                    tv = w1p.tile([128, D], F32R, tag="w1v")
                    for kk in range(ND):
                        nc.sync.dma_start(
                            tv[:, 128 * kk:128 * (kk + 1)],
                            w1_d.ap()[128 * kk:128 * (kk + 1),
                                      4 * D + 128 * j:4 * D + 128 * (j + 1)])
                    w1v[j] = tv
                    t2 = w2p.tile([128, D], F32R, tag="w2")
                    nc.sync.dma_start(t2[:],
                                      w2_d.ap()[128 * j:128 * (j + 1), :])
                    w2t[j] = t2
                for b in range(BL):
                    ajb = {}
                    for j in js:
                        pg = ps.tile([128, T], F32, tag="ps")
                        pv = ps.tile([128, T], F32, tag="ps")
                        for kk in range(ND):
                            _mm(nc, pg[:],
                                w1g[j][:, 128 * kk:128 * (kk + 1)],
                                h1nT[b][kk][:], kk == 0, kk == ND - 1)
                        for kk in range(ND):
                            _mm(nc, pv[:],
                                w1v[j][:, 128 * kk:128 * (kk + 1)],
                                h1nT[b][kk][:], kk == 0, kk == ND - 1)
                        sg = bsc.tile([128, T], F32, tag="sigB")
                        nc.scalar.activation(sg[:], pg[:], AF.Silu)
                        a = apb.tile([128, T], F32R, tag="aB")
                        nc.vector.tensor_tensor(a[:], sg[:], pv[:], ALU.mult)
                        ajb[j] = a
                    for jo in range(ND):
                        ph = ps.tile([128, T], F32, tag="ps")
                        for idx, j in enumerate(js):
                            _mm(nc, ph[:],
                                w2t[j][:, 128 * jo:128 * (jo + 1)],
                                ajb[j][:], idx == 0, idx == len(js) - 1)
                        nc.vector.tensor_tensor(h1T[b][jo][:], h1T[b][jo][:],
                                                ph[:], ALU.add)

        # ---------------- Phase C: memory ops + output ----------------
        with ExitStack() as cctx:
            wcp = cctx.enter_context(tc.tile_pool(name="wC", bufs=1))
            cp = cctx.enter_context(tc.tile_pool(name="actC", bufs=2))
            small = cctx.enter_context(tc.tile_pool(name="smC", bufs=2))

            wwk = [wcp.tile([128, D], F32R, tag=f"wwk{i}", name=f"wwk{i}") for i in range(ND)]
            wwv = [wcp.tile([128, D], F32R, tag=f"wwv{i}", name=f"wwv{i}") for i in range(ND)]
            wg1 = [wcp.tile([128, D], F32R, tag=f"wg1{i}", name=f"wg1{i}") for i in range(ND)]
            for i in range(ND):
                nc.sync.dma_start(wwk[i][:],
                                  wwk_d.ap()[128 * i:128 * (i + 1), :])
                nc.sync.dma_start(wwv[i][:],
                                  wwv_d.ap()[128 * i:128 * (i + 1), :])
                nc.sync.dma_start(wg1[i][:],
                                  wg1_d.ap()[128 * i:128 * (i + 1), :])

            for b in range(BL):
                h2 = h1T[b]
                # q_win cols (sum over t; 1/512 folded later)
                qwc = small.tile([128, ND], F32R, tag="qwc")
                for j in range(ND):
                    nc.vector.reduce_sum(qwc[:, j:j + 1], h2[j][:], axis=AX)
                # episodic tensors
                keysN = cp.tile([64, D], F32, tag="keysN")
                nc.sync.dma_start(keysN[:], keys_d.ap()[b])
                valsN = cp.tile([64, D], F32R, tag="valsN")
                nc.sync.dma_start(valsN[:], vals_d.ap()[b])
                keysTt = [cp.tile([128, K], F32R, tag=f"keysT{i}", name=f"keysTt{i}")
                          for i in range(ND)]
                for i in range(ND):
                    nc.sync.dma_start(keysTt[i][:],
                                      keysT_d.ap()[b, 128 * i:128 * (i + 1), :])
                age_r = small.tile([1, K], F32, tag="ager")
                nc.sync.dma_start(age_r[:], age_d.ap()[b:b + 1, :])
                str_r = small.tile([1, K], F32, tag="strr")
                nc.sync.dma_start(str_r[:], str_d.ap()[b:b + 1, :])

                # ||q_win|| -> rn scalar ; note qwc = 512*q_win
                pnq = ps.tile([1, 1], F32, tag="ps")
                for j in range(ND):
                    _mm(nc, pnq[:], qwc[:, j:j + 1], qwc[:, j:j + 1],
                        j == 0, j == ND - 1)
                nrm = small.tile([1, 1], F32, tag="nrm")
                ln_exp_scale(nrm[:], pnq[:], 1.0, 0.0, 0.5)  # = 512*||q_win||
                rn = small.tile([1, 1], F32, tag="rn")
                nc.vector.tensor_scalar(rn[:], nrm[:], 1.0 / D, 1e-6,
                                        ALU.mult, ALU.add)
                nc.vector.reciprocal(rn[:], rn[:])
                # keys row norms -> rk row
                kssq = small.tile([64, 1], F32, tag="kssq")
                scr = cp.tile([64, D], F32, tag="scrC")
                nc.vector.scalar_tensor_tensor(scr[:], keysN[:], 1.0,
                                               keysN[:], ALU.mult, ALU.mult,
                                               accum_out=kssq[:])
                knrm = small.tile([64, 1], F32, tag="knrm")
                ln_exp_scale(knrm[:], kssq[:], 1.0, 0.0, 0.5)
                nc.vector.tensor_scalar(knrm[:], knrm[:], 1e-6, None, ALU.add)
                nc.vector.reciprocal(knrm[:], knrm[:])
                prk = ps.tile([1, 64], F32, tag="ps")
                nc.tensor.transpose(prk[:], knrm[:], ident[0:64, 0:64])
                rk_r = small.tile([1, K], F32, tag="rkr")
                nc.vector.tensor_copy(rk_r[:], prk[:])
                # sim_r row
                psr = ps.tile([1, K], F32, tag="ps")
                for j in range(ND):
                    _mm(nc, psr[:], qwc[:, j:j + 1], keysTt[j][:],
                        j == 0, j == ND - 1)
                simr = small.tile([1, K], F32, tag="simr")
                nc.vector.tensor_scalar(simr[:], psr[:], rn[:], 1.0 / D,
                                        ALU.mult, ALU.mult)
                nc.vector.tensor_tensor(simr[:], simr[:], rk_r[:], ALU.mult)
                # reader logits
                lns = small.tile([1, K], F32, tag="lns")
                nc.vector.tensor_scalar(lns[:], str_r[:], 0.001, 1e9,
                                        ALU.max, ALU.min)
                nc.scalar.activation(lns[:], lns[:], AF.Ln)
                msk = small.tile([1, K], F32, tag="mskC")
                nc.vector.tensor_scalar(msk[:], str_r[:], 0.001, None,
                                        ALU.is_gt)
                nc.vector.tensor_scalar(msk[:], msk[:], 1.0, 1000.0,
                                        ALU.subtract, ALU.mult)
                lg = small.tile([1, K], F32, tag="lg")
                nc.vector.scalar_tensor_tensor(lg[:], lns[:], 0.5, simr[:],
                                               ALU.mult, ALU.add)
                nc.vector.scalar_tensor_tensor(lg[:], age_r[:], -0.02, lg[:],
                                               ALU.mult, ALU.add)
                nc.vector.tensor_tensor(lg[:], lg[:], msk[:], ALU.add)
                # softmax over K (row)
                negm = small.tile([1, 1], F32, tag="negm")
                nc.vector.reduce_max(negm[:], lg[:], axis=AX, negate=True)
                er = small.tile([1, K], F32, tag="er")
                nc.scalar.activation(er[:], lg[:], AF.Exp, bias=negm[:])
                ssum = small.tile([1, 1], F32, tag="ssum")
                nc.vector.reduce_sum(ssum[:], er[:], axis=AX)
                nc.vector.reciprocal(ssum[:], ssum[:])
                wread = small.tile([1, K], F32, tag="wread")
                nc.vector.tensor_scalar(wread[:], er[:], ssum[:], None,
                                        ALU.mult)
                pwc = ps.tile([64, 1], F32, tag="ps")
                nc.tensor.transpose(pwc[:], wread[:], ident[0:1, 0:1])
                wrc = small.tile([64, 1], F32R, tag="wrc")
                nc.vector.tensor_copy(wrc[:], pwc[:])
                prd = ps.tile([1, D], F32, tag="ps")
                _mm(nc, prd[:], wrc[:], valsN[:], True, True)
                # read = rms(read_raw) * rms_read
                rd0 = small.tile([1, D], F32, tag="rd0")
                nc.vector.tensor_copy(rd0[:], prd[:])
                rdsq = small.tile([1, 1], F32, tag="rdsq")
                rscr = small.tile([1, D], F32, tag="rscr")
                nc.vector.scalar_tensor_tensor(rscr[:], rd0[:], 1.0, rd0[:],
                                               ALU.mult, ALU.mult,
                                               accum_out=rdsq[:])
                rrd = small.tile([1, 1], F32, tag="rrd")
                ln_exp_scale(rrd[:], rdsq[:], 1.0 / D, 1e-6, -0.5)
                read_r = small.tile([1, D], F32, tag="readr")
                nc.vector.tensor_scalar(read_r[:], rd0[:], rrd[:], None,
                                        ALU.mult)
                nc.vector.tensor_tensor(read_r[:], read_r[:], rmsrd_r,
                                        ALU.mult)
                # write key/val rows
                pwk = ps.tile([1, D], F32, tag="ps")
                for j in range(ND):
                    _mm(nc, pwk[:], qwc[:, j:j + 1], wwk[j][:],
                        j == 0, j == ND - 1)
                wk_r = small.tile([1, D], F32R, tag="wkrow")
                nc.vector.scalar_tensor_tensor(wk_r[:], pwk[:], 1.0 / D,
                                               bwk_r, ALU.mult, ALU.add)
                pwv = ps.tile([1, D], F32, tag="ps")
                for j in range(ND):
                    _mm(nc, pwv[:], qwc[:, j:j + 1], wwv[j][:],
                        j == 0, j == ND - 1)
                wv_r = small.tile([1, D], F32R, tag="wvrow")
                nc.vector.scalar_tensor_tensor(wv_r[:], pwv[:], 1.0 / D,
                                               bwv_r, ALU.mult, ALU.add)
                # wk cols for norm + sim_w
                pkc = ps.tile([128, ND], F32, tag="ps")
                for j in range(ND):
                    nc.tensor.transpose(pkc[:, j:j + 1],
                                        wk_r[0:1, 128 * j:128 * (j + 1)]
                                        .bitcast(F32),
                                        ident[0:1, 0:1])
                wkc = small.tile([128, ND], F32R, tag="wkc")
                nc.vector.tensor_copy(wkc[:], pkc[:])
                pws = ps.tile([1, 1], F32, tag="ps")
                for j in range(ND):
                    _mm(nc, pws[:], wkc[:, j:j + 1], wkc[:, j:j + 1],
                        j == 0, j == ND - 1)
                rw = small.tile([1, 1], F32, tag="rw")
                ln_exp_scale(rw[:], pws[:], 1.0, 0.0, 0.5)
                nc.vector.tensor_scalar(rw[:], rw[:], 1e-6, None, ALU.add)
                nc.vector.reciprocal(rw[:], rw[:])
                psw = ps.tile([1, K], F32, tag="ps")
                for j in range(ND):
                    _mm(nc, psw[:], wkc[:, j:j + 1], keysTt[j][:],
                        j == 0, j == ND - 1)
                simw = small.tile([1, K], F32, tag="simw")
                nc.vector.tensor_scalar(simw[:], psw[:], rw[:], None, ALU.mult)
                nc.vector.tensor_tensor(simw[:], simw[:], rk_r[:], ALU.mult)
                # writer softmax*50, hard one-hot
                bs = small.tile([1, 1], F32, tag="bs")
                nc.vector.reduce_max(bs[:], simw[:], axis=AX)
                nb = small.tile([1, 1], F32, tag="nb")
                nc.vector.tensor_scalar(nb[:], bs[:], -50.0, None, ALU.mult)
                ew = small.tile([1, K], F32, tag="ew")
                nc.scalar.activation(ew[:], simw[:], AF.Exp, scale=50.0,
                                     bias=nb[:])
                wsum = small.tile([1, 1], F32, tag="wsum")
                nc.vector.reduce_sum(wsum[:], ew[:], axis=AX)
                nc.vector.reciprocal(wsum[:], wsum[:])
                soft = small.tile([1, K], F32, tag="soft")
                nc.vector.tensor_scalar(soft[:], ew[:], wsum[:], None,
                                        ALU.mult)
                smax = small.tile([1, 1], F32, tag="smax")
                nc.vector.reduce_max(smax[:], soft[:], axis=AX)
                hard = small.tile([1, K], F32, tag="hard")
                nc.vector.tensor_scalar(hard[:], soft[:], smax[:], None,
                                        ALU.is_ge)
                ww = small.tile([1, K], F32, tag="ww")
                nc.vector.tensor_tensor(ww[:], hard[:], soft[:], ALU.subtract)
                nc.vector.tensor_tensor(ww[:], ww[:], soft[:], ALU.add)
                # ws = sigmoid(q_win @ Wws + bws) via exp
                pss = ps.tile([1, 1], F32, tag="ps")
                for j in range(ND):
                    _mm(nc, pss[:], qwc[:, j:j + 1], wws_c[:, j:j + 1],
                        j == 0, j == ND - 1)
                wspre = small.tile([1, 1], F32, tag="wspre")
                nc.vector.scalar_tensor_tensor(wspre[:], pss[:], 1.0 / D,
                                               bws_s[:], ALU.mult, ALU.add)
                wss = small.tile([1, 1], F32, tag="wss")
                nc.scalar.activation(wss[:], wspre[:], AF.Exp, scale=-1.0)
                nc.vector.tensor_scalar(wss[:], wss[:], 1.0, None, ALU.add)
                nc.vector.reciprocal(wss[:], wss[:])
                # eff, cols
                eff = small.tile([1, K], F32, tag="eff")
                nc.vector.tensor_scalar(eff[:], ww[:], wss[:], 0.5,
                                        ALU.mult, ALU.mult)
                pec = ps.tile([64, 1], F32, tag="ps")
                nc.tensor.transpose(pec[:], eff[:], ident[0:1, 0:1])
                effc = small.tile([64, 1], F32, tag="effc")
                nc.vector.tensor_copy(effc[:], pec[:])
                omec = small.tile([64, 1], F32, tag="omec")
                nc.vector.tensor_scalar(omec[:], effc[:], -1.0, 1.0,
                                        ALU.mult, ALU.add)
                # keys_new
                pkb = ps.tile([64, D], F32, tag="ps")
                _mm(nc, pkb[:], ones[0:1, 0:64], wk_r[:], True, True)
                t1 = cp.tile([64, D], F32, tag="t1C")
                nc.vector.tensor_scalar(t1[:], keysN[:], omec[:], None,
                                        ALU.mult)
                kraw = cp.tile([64, D], F32, tag="krawC")
                nc.vector.scalar_tensor_tensor(kraw[:], pkb[:], effc[:],
                                               t1[:], ALU.mult, ALU.add)
                krssq = small.tile([64, 1], F32, tag="krssq")
                kscr = cp.tile([64, D], F32, tag="kscrC")
                nc.vector.scalar_tensor_tensor(kscr[:], kraw[:], 1.0, kraw[:],
                                               ALU.mult, ALU.mult,
                                               accum_out=krssq[:])
                krn = small.tile([64, 1], F32, tag="krn")
                ln_exp_scale(krn[:], krssq[:], 1.0, 0.0, 0.5)
                nc.vector.tensor_scalar(krn[:], krn[:], 1e-6, None, ALU.add)
                nc.vector.reciprocal(krn[:], krn[:])
                knew = cp.tile([64, D], F32, tag="knewC")
                nc.vector.tensor_scalar(knew[:], kraw[:], krn[:], None,
                                        ALU.mult)
                nc.sync.dma_start(kn_d.ap()[b], knew[:])
                # vals_new
                pvb = ps.tile([64, D], F32, tag="ps")
                _mm(nc, pvb[:], ones[0:1, 0:64], wv_r[:], True, True)
                t1v = cp.tile([64, D], F32, tag="t1vC")
                nc.vector.tensor_scalar(t1v[:], valsN[:], omec[:], None,
                                        ALU.mult)
                vnew = cp.tile([64, D], F32, tag="vnewC")
                nc.vector.scalar_tensor_tensor(vnew[:], pvb[:], effc[:],
                                               t1v[:], ALU.mult, ALU.add)
                nc.sync.dma_start(vn_d.ap()[b], vnew[:])
                # age_new, strength_new
                omw = small.tile([1, K], F32, tag="omw")
                nc.vector.tensor_scalar(omw[:], ww[:], -1.0, 1.0,
                                        ALU.mult, ALU.add)
                anew = small.tile([1, K], F32, tag="anew")
                nc.vector.scalar_tensor_tensor(anew[:], age_r[:], 1.0, omw[:],
                                               ALU.add, ALU.mult)
                nc.sync.dma_start(an_d.ap()[b:b + 1, :], anew[:])
                s995 = small.tile([1, K], F32, tag="s995")
                nc.vector.tensor_scalar(s995[:], str_r[:], 0.995, None,
                                        ALU.mult)
                oms = small.tile([1, K], F32, tag="oms")
                nc.vector.tensor_scalar(oms[:], s995[:], -1.0, 1.0,
                                        ALU.mult, ALU.add)
                wws2 = small.tile([1, K], F32, tag="wws2")
                nc.vector.tensor_scalar(wws2[:], eff[:], 2.0, None, ALU.mult)
                nc.vector.tensor_tensor(wws2[:], wws2[:], oms[:], ALU.mult)
                snew = small.tile([1, K], F32, tag="snew")
                nc.vector.tensor_tensor(snew[:], s995[:], wws2[:], ALU.add)
                nc.vector.tensor_scalar(snew[:], snew[:], 0.001, 1.0,
                                        ALU.max, ALU.min)
                nc.sync.dma_start(sn_d.ap()[b:b + 1, :], snew[:])
                # gate MLP
                pg1 = ps.tile([1, D], F32, tag="ps")
                for j in range(ND):
                    _mm(nc, pg1[:], qwc[:, j:j + 1], wg1[j][:],
                        j == 0, j == ND - 1)
                g1 = small.tile([1, D], F32, tag="g1")
                nc.vector.scalar_tensor_tensor(g1[:], pg1[:], 1.0 / D,
                                               bg1_r, ALU.mult, ALU.add)
                nov = small.tile([1, 1], F32, tag="nov")
                nc.vector.tensor_scalar(nov[:], bs[:], -1.0, 1.0,
                                        ALU.mult, ALU.add)
                wsnov_r = small.tile([1, 2], F32, tag="wsnovr")
                nc.vector.tensor_copy(wsnov_r[:, 0:1], wss[:])
                nc.vector.tensor_copy(wsnov_r[:, 1:2], nov[:])
                pwsn = ps.tile([2, 1], F32, tag="ps")
                nc.tensor.transpose(pwsn[:], wsnov_r[:], ident[0:1, 0:1])
                wsnov = small.tile([2, 1], F32R, tag="wsnov")
                nc.vector.tensor_copy(wsnov[:], pwsn[:])
                pg2 = ps.tile([1, D], F32, tag="ps")
                _mm(nc, pg2[:], wsnov[:], wg1t_r[:], True, True)
                nc.vector.tensor_tensor(g1[:], g1[:], pg2[:], ALU.add)
                # silu(g1) = g1 * sigmoid(g1) via exp
                eg = small.tile([1, D], F32, tag="eg")
                nc.scalar.activation(eg[:], g1[:], AF.Exp, scale=-1.0)
                nc.vector.tensor_scalar(eg[:], eg[:], 1.0, None, ALU.add)
                nc.vector.reciprocal(eg[:], eg[:])
                sg1 = small.tile([1, D], F32, tag="sg1")
                nc.vector.tensor_tensor(sg1[:], g1[:], eg[:], ALU.mult)
                psgc = ps.tile([128, ND], F32, tag="ps")
                for j in range(ND):
                    nc.tensor.transpose(psgc[:, j:j + 1],
                                        sg1[0:1, 128 * j:128 * (j + 1)],
                                        ident[0:1, 0:1])
                sgc = small.tile([128, ND], F32R, tag="sgc")
                nc.vector.tensor_copy(sgc[:], psgc[:])
                pgp = ps.tile([1, 1], F32, tag="ps")
                for j in range(ND):
                    _mm(nc, pgp[:], sgc[:, j:j + 1], wg2_c[:, j:j + 1],
                        j == 0, j == ND - 1)
                gpre = small.tile([1, 1], F32, tag="gpre")
                nc.vector.tensor_tensor(gpre[:], pgp[:], bg2_s[:], ALU.add)
                gat = small.tile([1, 1], F32, tag="gat")
                nc.scalar.activation(gat[:], gpre[:], AF.Exp, scale=-1.0)
                nc.vector.tensor_scalar(gat[:], gat[:], 1.0, None, ALU.add)
                nc.vector.reciprocal(gat[:], gat[:])
                nc.sync.dma_start(gate_d.ap()[b:b + 1, :], gat[:])
                # out = h2 + gate*read  (broadcast over t), then transpose out
                gr = small.tile([1, D], F32, tag="gr")
                nc.vector.tensor_scalar(gr[:], read_r[:], gat[:], None,
                                        ALU.mult)
                pgr = ps.tile([128, ND], F32, tag="ps")
                for j in range(ND):
                    nc.tensor.transpose(pgr[:, j:j + 1],
                                        gr[0:1, 128 * j:128 * (j + 1)],
                                        ident[0:1, 0:1])
                grc = small.tile([128, ND], F32, tag="grc")
                nc.vector.tensor_copy(grc[:], pgr[:])
                for j in range(ND):
                    nc.vector.tensor_scalar(h2[j][:], h2[j][:],
                                            grc[:, j:j + 1], None, ALU.add)
                for i in range(NT):
                    pot = ps.tile([128, D], F32, tag="ps")
                    for j in range(ND):
                        nc.tensor.transpose(pot[:, 128 * j:128 * (j + 1)],
                                            h2[j][:, 128 * i:128 * (i + 1)],
                                            ident[:])
                    ot = cp.tile([128, D], F32, tag="outN")
                    nc.scalar.copy(ot[:], pot[:])
                    nc.sync.dma_start(out_d.ap()[b, 128 * i:128 * (i + 1), :],
                                      ot[:])

    nc.compile()
    return nc


_NC_CACHE = {}


def _get_nc():
    if "nc" not in _NC_CACHE:
        _NC_CACHE["nc"] = _build()
    return _NC_CACHE["nc"]


def _get_runner():
    """Build the sharded jitted executable once (mirrors run_bass_via_pjrt)."""
    if "runner" in _NC_CACHE:
        return _NC_CACHE["runner"]
    import jax
    import concourse.mybir as mybir_
    from concourse import bass2jax
    from jax.experimental.shard_map import shard_map
    from jax.sharding import Mesh, PartitionSpec

    nc = _get_nc()
    bass2jax.install_neuronx_cc_hook()
    partition_name = (nc.partition_id_tensor.name
                      if nc.partition_id_tensor else None)
    in_names, out_names, out_avals, zero_shapes = [], [], [], []
    for alloc in nc.m.functions[0].allocations:
        if not isinstance(alloc, mybir_.MemoryLocationSet):
            continue
        name = alloc.memorylocations[0].name
        if alloc.kind == "ExternalInput":
            if name != partition_name:
                in_names.append(name)
        elif alloc.kind == "ExternalOutput":
            out_names.append(name)
            shape = tuple(alloc.tensor_shape)
            dtype = mybir_.dt.np(alloc.dtype)
            out_avals.append(jax.core.ShapedArray(shape, dtype))
            zero_shapes.append((shape, dtype))
    n_params = len(in_names)
    all_names = list(in_names) + list(out_names)
    if partition_name is not None:
        all_names.append(partition_name)
    donate = tuple(range(n_params, n_params + len(out_names)))

    def _body(*args):
        operands = list(args)
        if partition_name is not None:
            operands.append(bass2jax.partition_id_tensor())
        outs = bass2jax._bass_exec_p.bind(
            *operands,
            out_avals=tuple(out_avals),
            in_names=tuple(all_names),
            out_names=tuple(out_names),
            lowering_input_output_aliases=(),
            sim_require_finite=True,
            sim_require_nnan=True,
            nc=nc,
        )
        return tuple(outs)

    devices = jax.devices()[:NCORES]
    mesh = Mesh(np.asarray(devices), ("core",))
    n_out = len(out_names)
    in_specs = (PartitionSpec("core"),) * (n_params + n_out)
    out_specs = (PartitionSpec("core"),) * n_out
    fn = jax.jit(
        shard_map(_body, mesh=mesh, in_specs=in_specs, out_specs=out_specs,
                  check_rep=False),
        donate_argnums=donate, keep_unused=True)
    runner = {"fn": fn, "in_names": in_names, "out_names": out_names,
              "zero_shapes": zero_shapes, "mesh": mesh}
    _NC_CACHE["runner"] = runner
    return runner


def _concat_inputs(in_maps, runner):
    return [np.concatenate([np.asarray(m[n]) for m in in_maps], axis=0)
            for n in runner["in_names"]]


def _make_zeros(runner):
    return [np.zeros((NCORES * s[0], *s[1:]), d)
            for s, d in runner["zero_shapes"]]


def _split_outs(out_arrs, runner):
    res = [{} for _ in range(NCORES)]
    for i, n in enumerate(runner["out_names"]):
        arr = np.asarray(out_arrs[i])
        per = arr.shape[0] // NCORES
        for c in range(NCORES):
            res[c][n] = arr[c * per:(c + 1) * per]
    return res


def _swap_matrix():
    # psw = swapm.T @ q swaps the re/im 32-blocks within each 64-partition
    # head block (deinterleaved rope layout)
    sw = np.zeros((128, 128), np.float32)
    for a in (0, 64):
        for i in range(32):
            sw[a + 32 + i, a + i] = 1.0
            sw[a + i, a + 32 + i] = 1.0
    return sw


def _host_prep(inputs):
    f = lambda n: np.ascontiguousarray(np.asarray(inputs[n], np.float32))
    x = f("x")
    keys = f("epi_keys")
    vals = f("epi_vals")
    age = f("epi_age")
    strength = f("epi_strength")
    pos = np.asarray(inputs["pos_idx"]).astype(np.float64)

    # deinterleave perm per head: evens then odds
    ph = np.concatenate([np.arange(0, DH, 2), np.arange(1, DH, 2)])
    perm = np.concatenate([h * DH + ph for h in range(H)])
    wq_p = f("Wq")[:, perm]
    wk_p = f("Wk")[:, perm]
    bq_p = f("bq")[perm]
    bk_p = f("bk")[perm]

    freqs = 1.0 / (10000.0 ** (np.arange(0, DH, 2, dtype=np.float64) / DH))
    ang = pos[None, :] * freqs[:, None]          # [32, T]
    cos32 = np.cos(ang).astype(np.float32)
    sin32 = np.sin(ang).astype(np.float32)
    cosT = np.empty((128, T), np.float32)
    sinT = np.empty((128, T), np.float32)
    for blk in range(2):
        o = blk * 64
        cosT[o:o + 32] = cos32
        cosT[o + 32:o + 64] = cos32
        sinT[o:o + 32] = -sin32
        sinT[o + 32:o + 64] = sin32

    mask = np.triu(np.ones((T, T), np.float32))  # mask[s,t] = 1 if s<=t

    common = {
        "Wq": wq_p, "Wk": wk_p, "Wv": f("Wv"), "Wo": f("Wo"),
        "bq": bq_p, "bk": bk_p,
        "bv_bcast": np.tile(f("bv")[None, :], (128, 1)),
        "rms2_bcast": np.tile(f("rms2")[None, :], (128, 1)),
        "bo": f("bo"), "W1": f("W1"), "W2": f("W2"),
        "Wwk": f("Wwk"), "Wwv": f("Wwv"),
        "bwk": f("bwk"), "bwv": f("bwv"),
        "Wws": f("Wws").reshape(D), "bws": f("bws").reshape(1, 1),
        "Wg1": f("Wg1"), "bg1": f("bg1").reshape(1, D),
        "Wg2": f("Wg2").reshape(D), "bg2": f("bg2").reshape(1, 1),
        "rms1": f("rms1"), "rms_kv": f("rms_kv"), "rms2": f("rms2"),
        "rms_read": f("rms_read").reshape(1, D),
        "rope_cos": cosT, "rope_sin": sinT, "mask": mask,
        "ident": np.eye(128, dtype=np.float32),
        "ones": np.ones((128, 128), np.float32),
        "swapm": _swap_matrix(),
    }
    keysT = np.ascontiguousarray(keys.transpose(0, 2, 1))
    in_maps = []
    for c in range(NCORES):
        s = slice(c * BL, (c + 1) * BL)
        m = dict(common)
        m.update({"x": x[s], "keys": keys[s], "keysT": keysT[s],
                  "vals": vals[s], "age": age[s], "strength": strength[s]})
        in_maps.append(m)
    return in_maps


def kernel(**inputs):
    runner = _get_runner()
    in_maps = _host_prep(inputs)
    out_arrs = runner["fn"](*_concat_inputs(in_maps, runner),
                            *_make_zeros(runner))
    res = _split_outs(out_arrs, runner)
    cat = lambda n: np.concatenate([res[c][n] for c in range(NCORES)], axis=0)
    return (cat("out"), cat("keys_new"), cat("vals_new"), cat("age_new"),
            cat("strength_new"), cat("gate"))


def timed_run(inputs, iters=8):
    """Device-resident timing: returns (per_call_seconds_list, results)."""
    import time as _time
    import jax
    runner = _get_runner()
    in_maps = _host_prep(inputs)
    dev_ins = [jax.device_put(a) for a in _concat_inputs(in_maps, runner)]
    fn = runner["fn"]
    zero_sets = [[jax.device_put(z) for z in _make_zeros(runner)]
                 for _ in range(iters + 1)]
    for zs in zero_sets:
        jax.block_until_ready(zs)
    # warmup
    out = fn(*dev_ins, *zero_sets[0])
    jax.block_until_ready(out)
    times = []
    for it in range(iters):
        t0 = _time.perf_counter()
        out = fn(*dev_ins, *zero_sets[it + 1])
        jax.block_until_ready(out)
        times.append(_time.perf_counter() - t0)
    return times, out
